# revision 2
# baseline (speedup 1.0000x reference)
"""2-layer GCN + JumpingKnowledge(cat) + Linear on 8 Trainium2 NeuronCores.

Strategy (graph-parallel, nodes sharded 6250/core):
  - g = dinv * (x @ W) computed per-core (TensorE + DVE), written to HBM,
    AllGather'd so every core holds the full node-feature table.
  - Message passing per destination tile (128 dsts): dma_gather pulls the
    unique source rows (fp32, 512B each) from the table; aggregation is a
    chain of TensorE matmuls  psum[feat,dst] += M_chunk^T @ S_chunk  where
    S (host-built, fp32) carries the symmetric-norm coefficients dinv[dst]
    (multiplicity-summed, dedup'd sources).  Self-loops use the local g
    tile against diag(dinv) - no gather needed.
  - relu(+bias) on ScalarE writes the transposed activations x^T directly,
    which feeds the next layer's matmuls without any transposes.
  - Final: out = x1 @ lin_W[:D] + x2 @ lin_W[D:] (+ lin_b) per tile.

dma_gather requires int16 indices, so the gather table is addressed as two
halves (rows [0, 25088) and [25088, 50176)), with per-tile chunk counts
shared across cores (SPMD uniformity).
"""
import math
import numpy as np

import concourse.bass as bass
import concourse.bacc as bacc
import concourse.mybir as mybir
import concourse.tile as tile
from concourse._compat import get_trn_type
from concourse.bass_utils import run_bass_kernel_spmd
from concourse.library_config import mlp
from concourse.masks import make_identity

P = 128
N_CORES = 8

f32 = mybir.dt.float32
i16 = mybir.dt.int16


def _preprocess(x, edge_index, lin_W, lin_b, b1, b2):
    """Host-side (numpy, integer/structural + GCN-standard norm precompute)."""
    N, D = x.shape
    assert D == P
    E = edge_index.shape[1]
    C = N_CORES
    NPC = (N + C - 1) // C
    assert NPC * C == N, "node count must split evenly across cores"
    NPAD = ((NPC + P - 1) // P) * P
    T = NPAD // P
    FULL = C * NPAD
    HC = C // 2
    HALF_ROW = HC * NPAD
    HALF_NODE = HC * NPC
    assert HALF_ROW <= 32768 and FULL - HALF_ROW <= 32768

    src = edge_index[0].astype(np.int64)
    dst = edge_index[1].astype(np.int64)

    deg = np.bincount(dst, minlength=N).astype(np.float32) + 1.0
    dinv = (1.0 / np.sqrt(deg)).astype(np.float32)

    table_row = (src // NPC) * NPAD + (src % NPC)
    half = (table_row >= HALF_ROW).astype(np.int64)
    row_in_half = table_row - half * HALF_ROW
    core = dst // NPC
    dloc = dst % NPC
    tl = dloc // P
    dcol = dloc % P
    vals = dinv[dst]

    # group edges by (core, tile, half), rows sorted within group
    gkey = (core * T + tl) * 2 + half
    order = np.lexsort((row_in_half, gkey))
    gkey_s = gkey[order]
    rows_s = row_in_half[order]
    dcol_s = dcol[order]
    vals_s = vals[order]
    n_groups = C * T * 2
    bounds = np.searchsorted(gkey_s, np.arange(n_groups + 1))

    # per-group unique rows
    uniq = [None] * n_groups
    invs = [None] * n_groups
    for g in range(n_groups):
        lo, hi = bounds[g], bounds[g + 1]
        r = rows_s[lo:hi]
        u = np.unique(r)
        uniq[g] = u
        invs[g] = np.searchsorted(u, r)

    # SPMD-uniform chunk schedule: max over cores per (tile, half)
    ch = np.zeros((C, T, 2), np.int64)
    for g in range(n_groups):
        c, rem = divmod(g, T * 2)
        t, h = divmod(rem, 2)
        ch[c, t, h] = (len(uniq[g]) + P - 1) // P
    sched = np.maximum(ch.max(axis=0), 1)  # [T, 2]
    SCH = int(sched.sum())  # total chunks per core per layer

    # chunk offset of (t, h) in the concatenated per-core arrays
    flat_sched = sched.reshape(-1)
    chunk_off = np.zeros(T * 2 + 1, np.int64)
    np.cumsum(flat_sched, out=chunk_off[1:])

    per_core = []
    for c in range(C):
        idx_cols = np.zeros((P, SCH * 8), np.int16)
        flat_rows = np.zeros(E // C + 4 * SCH * P, np.int64)  # (goff, dcol, val) via bincount
        flat_dcol = np.zeros_like(flat_rows)
        flat_vals = np.zeros(len(flat_rows), np.float32)
        n_e = 0
        for t in range(T):
            for h in range(2):
                g = (c * T + t) * 2 + h
                nch = int(sched[t, h])
                K = nch * P
                u = uniq[g]
                u_pad = np.zeros(K, np.int64)
                u_pad[: len(u)] = u
                # int16 wrap: flat i -> (partition i%16, col i//16), replicated x8
                wrapped = u_pad.astype(np.int16).reshape(nch * 8, 16).T  # [16, nch*8]
                co = int(chunk_off[t * 2 + h])
                idx_cols[:, co * 8 : (co + nch) * 8] = np.tile(wrapped, (8, 1))
                lo, hi = bounds[g], bounds[g + 1]
                ne = hi - lo
                flat_rows[n_e : n_e + ne] = co * P + invs[g]
                flat_dcol[n_e : n_e + ne] = dcol_s[lo:hi]
                flat_vals[n_e : n_e + ne] = vals_s[lo:hi]
                n_e += ne
        flat = flat_rows[:n_e] * P + flat_dcol[:n_e]
        s_core = np.bincount(flat, weights=flat_vals[:n_e], minlength=SCH * P * P)
        s_core = s_core.astype(np.float32).reshape(SCH, P, P)
        smat = np.ascontiguousarray(s_core.transpose(1, 0, 2)).reshape(P, SCH * P)

        xT = np.zeros((P, NPAD), np.float32)
        xT[:, :NPC] = x[c * NPC : (c + 1) * NPC].T
        dv = np.zeros(NPAD, np.float32)
        dv[:NPC] = dinv[c * NPC : (c + 1) * NPC]
        dinv_tiles = np.ascontiguousarray(dv.reshape(T, P).T)  # [P, T]

        per_core.append(
            {
                "xT": xT,
                "dinv": dinv_tiles,
                "idx": idx_cols,
                "smat": smat,
            }
        )

    plan = {
        "N": N, "D": D, "E": E, "C": C, "NPC": NPC, "NPAD": NPAD, "T": T,
        "FULL": FULL, "HALF_ROW": HALF_ROW, "SCH": SCH,
        "sched": sched, "chunk_off": chunk_off,
    }
    return plan, per_core


def _build(plan):
    T = plan["T"]
    NPAD = plan["NPAD"]
    FULL = plan["FULL"]
    HALF_ROW = plan["HALF_ROW"]
    SCH = plan["SCH"]
    sched = plan["sched"]
    chunk_off = plan["chunk_off"]
    maxch = int(sched.sum(axis=1).max())

    nc = bacc.Bacc(
        get_trn_type() or "TRN2",
        target_bir_lowering=False,
        debug=False,
        num_devices=N_CORES,
    )
    xT_in = nc.dram_tensor("xT", [P, NPAD], f32, kind="ExternalInput").ap()
    w1_in = nc.dram_tensor("w1", [P, P], f32, kind="ExternalInput").ap()
    w2_in = nc.dram_tensor("w2", [P, P], f32, kind="ExternalInput").ap()
    lin1_in = nc.dram_tensor("lin1", [P, P], f32, kind="ExternalInput").ap()
    lin2_in = nc.dram_tensor("lin2", [P, P], f32, kind="ExternalInput").ap()
    b1_in = nc.dram_tensor("b1", [P, 1], f32, kind="ExternalInput").ap()
    b2_in = nc.dram_tensor("b2", [P, 1], f32, kind="ExternalInput").ap()
    linb_in = nc.dram_tensor("linb", [P, P], f32, kind="ExternalInput").ap()
    dinv_in = nc.dram_tensor("dinv", [P, T], f32, kind="ExternalInput").ap()
    idx_in = nc.dram_tensor("idx", [P, SCH * 8], i16, kind="ExternalInput").ap()
    smat_in = nc.dram_tensor("smat", [P, SCH * P], f32, kind="ExternalInput").ap()
    out_ap = nc.dram_tensor("out", [NPAD, P], f32, kind="ExternalOutput").ap()
    out_v = out_ap.rearrange("(t p) f -> p t f", p=P)

    nc.gpsimd.load_library(mlp)

    with tile.TileContext(nc) as tc:
        with (
            tc.tile_pool(name="dram", bufs=1, space="DRAM") as dram,
            tc.tile_pool(name="consts", bufs=1) as consts,
            tc.tile_pool(name="xTp", bufs=1) as xTp,
            tc.tile_pool(name="gstage", bufs=1) as gstagep,
            tc.tile_pool(name="x1Tp", bufs=1) as x1Tp,
            tc.tile_pool(name="x2Tp", bufs=1) as x2Tp,
            tc.tile_pool(name="msg", bufs=2) as msgp,
            tc.tile_pool(name="smat", bufs=2) as smatp,
            tc.tile_pool(name="diag", bufs=3) as diagp,
            tc.tile_pool(name="otile", bufs=3) as otilep,
            tc.tile_pool(name="ps_phase", bufs=4, space="PSUM") as psphase,
            tc.tile_pool(name="ps_agg", bufs=4, space="PSUM") as psagg,
        ):
            # ---- constants / inputs to SBUF
            xT = xTp.tile([P, NPAD], f32)
            nc.sync.dma_start(xT[:], xT_in[:])
            w1 = consts.tile([P, P], f32, tag="w1")
            nc.sync.dma_start(w1[:], w1_in[:])
            w2 = consts.tile([P, P], f32, tag="w2")
            nc.sync.dma_start(w2[:], w2_in[:])
            lin1 = consts.tile([P, P], f32, tag="lin1")
            nc.sync.dma_start(lin1[:], lin1_in[:])
            lin2 = consts.tile([P, P], f32, tag="lin2")
            nc.sync.dma_start(lin2[:], lin2_in[:])
            b1 = consts.tile([P, 1], f32, tag="b1")
            nc.sync.dma_start(b1[:], b1_in[:])
            b2 = consts.tile([P, 1], f32, tag="b2")
            nc.sync.dma_start(b2[:], b2_in[:])
            linb = consts.tile([P, P], f32, tag="linb")
            nc.sync.dma_start(linb[:], linb_in[:])
            dinv = consts.tile([P, T], f32, tag="dinv")
            nc.sync.dma_start(dinv[:], dinv_in[:])
            idx_sb = consts.tile([P, SCH * 8], i16, tag="idx")
            nc.sync.dma_start(idx_sb[:], idx_in[:])
            ident = consts.tile([P, P], f32, tag="ident")
            make_identity(nc, ident[:])

            gstage = gstagep.tile([P, NPAD], f32)
            x1T = x1Tp.tile([P, NPAD], f32)
            x2T = x2Tp.tile([P, NPAD], f32)

            g_loc = [
                dram.tile([NPAD, P], f32, tag=f"gloc{i}", name=f"gloc{i}")
                for i in range(2)
            ]
            g_full = [
                dram.tile([FULL, P], f32, tag=f"gfull{i}", name=f"gfull{i}")
                for i in range(2)
            ]

            def phase_g(src_stage, w_tile, layer):
                for t in range(T):
                    ps = psphase.tile([P, P], f32, tag="ps_phase")
                    nc.tensor.matmul(
                        ps[:], lhsT=src_stage[:, bass.ts(t, P)], rhs=w_tile[:],
                        start=True, stop=True,
                    )
                    nc.vector.tensor_scalar(
                        gstage[:, bass.ts(t, P)], ps[:],
                        dinv[:, t : t + 1], None, mybir.AluOpType.mult,
                    )
                gl = g_loc[layer]
                nc.sync.dma_start(
                    gl[:].rearrange("(t p) f -> p t f", p=P), gstage[:]
                )
                nc.gpsimd.collective_compute(
                    "AllGather",
                    mybir.AluOpType.bypass,
                    replica_groups=[list(range(N_CORES))],
                    ins=[gl.opt()],
                    outs=[g_full[layer].opt()],
                )

            def layer_agg(layer, xout, bias):
                gf = g_full[layer][:]
                half_views = [gf[0:HALF_ROW, :], gf[HALF_ROW:FULL, :]]
                for t in range(T):
                    nch = [int(sched[t, 0]), int(sched[t, 1])]
                    tot = nch[0] + nch[1]
                    msg = msgp.tile([P, maxch, P], f32, tag="msg")
                    s_sb = smatp.tile([P, maxch * P], f32, tag="smat")
                    co = int(chunk_off[t * 2])
                    nc.sync.dma_start(
                        s_sb[:, 0 : tot * P], smat_in[:, co * P : (co + tot) * P]
                    )
                    jo = 0
                    for h in range(2):
                        K = nch[h] * P
                        coh = int(chunk_off[t * 2 + h])
                        nc.gpsimd.dma_gather(
                            msg[:, jo : jo + nch[h], :],
                            half_views[h],
                            idx_sb[:, coh * 8 : coh * 8 + nch[h] * 8],
                            K, K, P,
                            single_packet=False,
                        )
                        jo += nch[h]
                    diag = diagp.tile([P, P], f32, tag="diag")
                    nc.vector.tensor_scalar(
                        diag[:], ident[:], dinv[:, t : t + 1], None,
                        mybir.AluOpType.mult,
                    )
                    ps = psagg.tile([P, P], f32, tag="ps_agg")
                    for j in range(tot):
                        nc.tensor.matmul(
                            ps[:], lhsT=msg[:, j, :],
                            rhs=s_sb[:, bass.ts(j, P)],
                            start=(j == 0), stop=False,
                        )
                    nc.tensor.matmul(
                        ps[:], lhsT=gstage[:, bass.ts(t, P)], rhs=diag[:],
                        start=(tot == 0), stop=True,
                    )
                    nc.scalar.activation(
                        xout[:, bass.ts(t, P)], ps[:],
                        mybir.ActivationFunctionType.Relu, bias=bias[:],
                    )

            phase_g(xT, w1, 0)
            layer_agg(0, x1T, b1)
            phase_g(x1T, w2, 1)
            layer_agg(1, x2T, b2)

            for t in range(T):
                ps = psphase.tile([P, P], f32, tag="ps_phase")
                nc.tensor.matmul(
                    ps[:], lhsT=x1T[:, bass.ts(t, P)], rhs=lin1[:],
                    start=True, stop=False,
                )
                nc.tensor.matmul(
                    ps[:], lhsT=x2T[:, bass.ts(t, P)], rhs=lin2[:],
                    start=False, stop=True,
                )
                ot = otilep.tile([P, P], f32, tag="otile")
                nc.vector.tensor_tensor(
                    out=ot[:], in0=ps[:], in1=linb[:], op=mybir.AluOpType.add
                )
                nc.sync.dma_start(out_v[:, t, :], ot[:])

    nc.compile()
    return nc


def kernel(x, edge_index, W1, b1, W2, b2, lin_W, lin_b):
    x = np.asarray(x, np.float32)
    edge_index = np.asarray(edge_index)
    W1 = np.asarray(W1, np.float32)
    W2 = np.asarray(W2, np.float32)
    b1 = np.asarray(b1, np.float32)
    b2 = np.asarray(b2, np.float32)
    lin_W = np.asarray(lin_W, np.float32)
    lin_b = np.asarray(lin_b, np.float32)

    plan, per_core = _preprocess(x, edge_index, lin_W, lin_b, b1, b2)
    nc = _build(plan)

    N, D, C, NPC, NPAD = plan["N"], plan["D"], plan["C"], plan["NPC"], plan["NPAD"]
    in_maps = []
    for c in range(C):
        pc = per_core[c]
        in_maps.append(
            {
                "xT": pc["xT"],
                "w1": W1,
                "w2": W2,
                "lin1": np.ascontiguousarray(lin_W[:D]),
                "lin2": np.ascontiguousarray(lin_W[D:]),
                "b1": b1[:, None].astype(np.float32),
                "b2": b2[:, None].astype(np.float32),
                "linb": np.tile(lin_b, (P, 1)).astype(np.float32),
                "dinv": pc["dinv"],
                "idx": pc["idx"],
                "smat": pc["smat"],
            }
        )

    last_err = None
    for _attempt in range(3):
        try:
            res = run_bass_kernel_spmd(nc, in_maps, list(range(C)))
            break
        except Exception as e:  # transient NRT device wedges happen
            last_err = e
    else:
        raise last_err

    out = np.empty((N, D), np.float32)
    for c in range(C):
        out[c * NPC : (c + 1) * NPC] = res.results[c]["out"][:NPC]
    return out


# revision 3
# speedup vs baseline: 1.0111x; 1.0111x over previous
"""2-layer GCN + JumpingKnowledge(cat) + Linear on 8 Trainium2 NeuronCores.

Strategy (graph-parallel, nodes sharded 6250/core):
  - g = dinv * (x @ W) computed per-core (TensorE + DVE), written to HBM,
    AllGather'd so every core holds the full node-feature table.
  - Message passing per destination tile (128 dsts): dma_gather pulls the
    unique source rows (fp32, 512B each) from the table; aggregation is a
    chain of TensorE matmuls  psum[feat,dst] += M_chunk^T @ S_chunk  where
    S (host-built, fp32) carries the symmetric-norm coefficients dinv[dst]
    (multiplicity-summed, dedup'd sources).  Self-loops use the local g
    tile against diag(dinv) - no gather needed.
  - relu(+bias) on ScalarE writes the transposed activations x^T directly,
    which feeds the next layer's matmuls without any transposes.
  - Final: out = x1 @ lin_W[:D] + x2 @ lin_W[D:] (+ lin_b) per tile.

dma_gather needs int16 indices, so the global table is built as two block
tables (each core contributes rows [0,3200) to table A and [3200,6272) to
table B).  Each block is AllGather'd separately so the collectives overlap
the gather stream of the previous work (the single Q7 descriptor-generation
core is the critical path at ~8.4ns/row; everything else hides under it).
"""
import numpy as np

import concourse.bass as bass
import concourse.bacc as bacc
import concourse.mybir as mybir
import concourse.tile as tile
from concourse._compat import get_trn_type
from concourse.bass_utils import run_bass_kernel_spmd
from concourse.library_config import mlp
from concourse.masks import make_identity

P = 128
N_CORES = 8

f32 = mybir.dt.float32
i16 = mybir.dt.int16


def _preprocess(x, edge_index):
    """Host-side (numpy): shard, block-split, dedup, build gather indices
    and the norm-coefficient matrices S."""
    N, D = x.shape
    assert D == P
    E = edge_index.shape[1]
    C = N_CORES
    NPC = (N + C - 1) // C
    assert NPC * C == N, "node count must split evenly across cores"
    NPAD = ((NPC + P - 1) // P) * P
    T = NPAD // P
    TA = (T + 1) // 2          # tiles in block A
    TB = T - TA
    BLKA, BLKB = TA * P, TB * P
    FULLA, FULLB = C * BLKA, C * BLKB
    assert FULLA <= 32768 and FULLB <= 32768

    src = edge_index[0].astype(np.int64)
    dst = edge_index[1].astype(np.int64)

    deg = np.bincount(dst, minlength=N).astype(np.float32) + 1.0
    dinv = (1.0 / np.sqrt(deg)).astype(np.float32)

    off = src % NPC
    blk = (off >= BLKA).astype(np.int64)
    row_in_blk = np.where(blk == 0, (src // NPC) * BLKA + off,
                          (src // NPC) * BLKB + (off - BLKA))
    core = dst // NPC
    dloc = dst % NPC
    tl = dloc // P
    dcol = dloc % P
    vals = dinv[dst]

    gkey = (core * T + tl) * 2 + blk
    order = np.lexsort((row_in_blk, gkey))
    gkey_s = gkey[order]
    rows_s = row_in_blk[order]
    dcol_s = dcol[order]
    vals_s = vals[order]
    n_groups = C * T * 2
    bounds = np.searchsorted(gkey_s, np.arange(n_groups + 1))

    uniq = [None] * n_groups
    invs = [None] * n_groups
    for g in range(n_groups):
        lo, hi = bounds[g], bounds[g + 1]
        r = rows_s[lo:hi]
        u = np.unique(r)
        uniq[g] = u
        invs[g] = np.searchsorted(u, r)

    # SPMD-uniform chunk schedule: max over cores per (tile, half)
    ch = np.zeros((C, T, 2), np.int64)
    for g in range(n_groups):
        c, rem = divmod(g, T * 2)
        t, h = divmod(rem, 2)
        ch[c, t, h] = (len(uniq[g]) + P - 1) // P
    sched = np.maximum(ch.max(axis=0), 1)  # [T, 2]
    SCH = int(sched.sum())

    flat_sched = sched.reshape(-1)
    chunk_off = np.zeros(T * 2 + 1, np.int64)
    np.cumsum(flat_sched, out=chunk_off[1:])

    per_core = []
    for c in range(C):
        idx_cols = np.zeros((P, SCH * 8), np.int16)
        cap = E // C + 4 * SCH * P + E // 16
        flat_rows = np.zeros(cap, np.int64)
        flat_dcol = np.zeros(cap, np.int64)
        flat_vals = np.zeros(cap, np.float32)
        n_e = 0
        for t in range(T):
            for h in range(2):
                g = (c * T + t) * 2 + h
                nch = int(sched[t, h])
                K = nch * P
                u = uniq[g]
                u_pad = np.zeros(K, np.int64)
                u_pad[: len(u)] = u
                wrapped = u_pad.astype(np.int16).reshape(nch * 8, 16).T
                co = int(chunk_off[t * 2 + h])
                idx_cols[:, co * 8 : (co + nch) * 8] = np.tile(wrapped, (8, 1))
                lo, hi = bounds[g], bounds[g + 1]
                ne = hi - lo
                flat_rows[n_e : n_e + ne] = co * P + invs[g]
                flat_dcol[n_e : n_e + ne] = dcol_s[lo:hi]
                flat_vals[n_e : n_e + ne] = vals_s[lo:hi]
                n_e += ne
        flat = flat_rows[:n_e] * P + flat_dcol[:n_e]
        s_core = np.bincount(flat, weights=flat_vals[:n_e], minlength=SCH * P * P)
        s_core = s_core.astype(np.float32).reshape(SCH, P, P)
        smat = np.ascontiguousarray(s_core.transpose(1, 0, 2)).reshape(P, SCH * P)

        xT = np.zeros((P, NPAD), np.float32)
        xT[:, :NPC] = x[c * NPC : (c + 1) * NPC].T
        dv = np.zeros(NPAD, np.float32)
        dv[:NPC] = dinv[c * NPC : (c + 1) * NPC]
        dinv_tiles = np.ascontiguousarray(dv.reshape(T, P).T)  # [P, T]

        per_core.append({"xT": xT, "dinv": dinv_tiles, "idx": idx_cols, "smat": smat})

    plan = {
        "N": N, "D": D, "E": E, "C": C, "NPC": NPC, "NPAD": NPAD, "T": T,
        "TA": TA, "TB": TB, "BLKA": BLKA, "BLKB": BLKB,
        "FULLA": FULLA, "FULLB": FULLB, "SCH": SCH,
        "sched": sched, "chunk_off": chunk_off,
    }
    return plan, per_core


def _build(plan):
    T, TA, TB = plan["T"], plan["TA"], plan["TB"]
    NPAD = plan["NPAD"]
    BLKA, BLKB = plan["BLKA"], plan["BLKB"]
    FULLA, FULLB = plan["FULLA"], plan["FULLB"]
    SCH = plan["SCH"]
    sched = plan["sched"]
    chunk_off = plan["chunk_off"]
    maxch = int(sched.sum(axis=1).max())

    nc = bacc.Bacc(
        get_trn_type() or "TRN2",
        target_bir_lowering=False,
        debug=False,
        num_devices=N_CORES,
    )
    xT_in = nc.dram_tensor("xT", [P, NPAD], f32, kind="ExternalInput").ap()
    w1_in = nc.dram_tensor("w1", [P, P], f32, kind="ExternalInput").ap()
    w2_in = nc.dram_tensor("w2", [P, P], f32, kind="ExternalInput").ap()
    lin1_in = nc.dram_tensor("lin1", [P, P], f32, kind="ExternalInput").ap()
    lin2_in = nc.dram_tensor("lin2", [P, P], f32, kind="ExternalInput").ap()
    b1_in = nc.dram_tensor("b1", [P, 1], f32, kind="ExternalInput").ap()
    b2_in = nc.dram_tensor("b2", [P, 1], f32, kind="ExternalInput").ap()
    linb_in = nc.dram_tensor("linb", [P, P], f32, kind="ExternalInput").ap()
    dinv_in = nc.dram_tensor("dinv", [P, T], f32, kind="ExternalInput").ap()
    idx_in = nc.dram_tensor("idx", [P, SCH * 8], i16, kind="ExternalInput").ap()
    smat_in = nc.dram_tensor("smat", [P, SCH * P], f32, kind="ExternalInput").ap()
    out_ap = nc.dram_tensor("out", [NPAD, P], f32, kind="ExternalOutput").ap()
    out_v = out_ap.rearrange("(t p) f -> p t f", p=P)

    nc.gpsimd.load_library(mlp)

    with tile.TileContext(nc) as tc:
        with (
            tc.tile_pool(name="dram", bufs=1, space="DRAM") as dram,
            tc.tile_pool(name="consts", bufs=1) as consts,
            tc.tile_pool(name="xTp", bufs=1) as xTp,
            tc.tile_pool(name="stages", bufs=1) as stages,
            tc.tile_pool(name="msg", bufs=3) as msgp,
            tc.tile_pool(name="smat", bufs=3) as smatp,
            tc.tile_pool(name="diag", bufs=3) as diagp,
            tc.tile_pool(name="otile", bufs=3) as otilep,
            tc.tile_pool(name="ps_phase", bufs=4, space="PSUM") as psphase,
            tc.tile_pool(name="ps_agg", bufs=4, space="PSUM") as psagg,
        ):
            xT = xTp.tile([P, NPAD], f32)
            nc.sync.dma_start(xT[:], xT_in[:])
            w1 = consts.tile([P, P], f32, tag="w1")
            nc.sync.dma_start(w1[:], w1_in[:])
            w2 = consts.tile([P, P], f32, tag="w2")
            nc.sync.dma_start(w2[:], w2_in[:])
            lin1 = consts.tile([P, P], f32, tag="lin1")
            nc.sync.dma_start(lin1[:], lin1_in[:])
            lin2 = consts.tile([P, P], f32, tag="lin2")
            nc.sync.dma_start(lin2[:], lin2_in[:])
            b1 = consts.tile([P, 1], f32, tag="b1")
            nc.sync.dma_start(b1[:], b1_in[:])
            b2 = consts.tile([P, 1], f32, tag="b2")
            nc.sync.dma_start(b2[:], b2_in[:])
            linb = consts.tile([P, P], f32, tag="linb")
            nc.sync.dma_start(linb[:], linb_in[:])
            dinv = consts.tile([P, T], f32, tag="dinv")
            nc.sync.dma_start(dinv[:], dinv_in[:])
            idx_sb = consts.tile([P, SCH * 8], i16, tag="idx")
            nc.sync.dma_start(idx_sb[:], idx_in[:])
            ident = consts.tile([P, P], f32, tag="ident")
            make_identity(nc, ident[:])

            # per-block stage tiles: [P, BLKA] and [P, BLKB]
            def blk_pair(tag):
                a = stages.tile([P, BLKA], f32, tag=f"{tag}A", name=f"{tag}A")
                b = stages.tile([P, BLKB], f32, tag=f"{tag}B", name=f"{tag}B")
                return [a, b]

            gstage = blk_pair("gstage")
            x1T = blk_pair("x1T")
            x2T = blk_pair("x2T")

            g_loc = [[None, None], [None, None]]
            g_full = [[None, None], [None, None]]
            for layer in range(2):
                for h, (blkrows, fullrows) in enumerate([(BLKA, FULLA), (BLKB, FULLB)]):
                    g_loc[layer][h] = dram.tile(
                        [blkrows, P], f32, tag=f"gloc{layer}{h}", name=f"gloc{layer}{h}"
                    )
                    g_full[layer][h] = dram.tile(
                        [fullrows, P], f32, tag=f"gfull{layer}{h}", name=f"gfull{layer}{h}"
                    )

            def loc_tile(t):
                """(stage-half h, column-tile index within that half)"""
                return (0, t) if t < TA else (1, t - TA)

            def phase_g_block(src_stages, w_tile, layer, h):
                t0 = 0 if h == 0 else TA
                nt = TA if h == 0 else TB
                gs = gstage[h]
                for i in range(nt):
                    t = t0 + i
                    hh, ii = loc_tile(t)
                    ps = psphase.tile([P, P], f32, tag="ps_phase", name="psph")
                    nc.tensor.matmul(
                        ps[:], lhsT=src_stages[hh][:, bass.ts(ii, P)], rhs=w_tile[:],
                        start=True, stop=True,
                    )
                    nc.vector.tensor_scalar(
                        gs[:, bass.ts(i, P)], ps[:],
                        dinv[:, t : t + 1], None, mybir.AluOpType.mult,
                    )
                gl = g_loc[layer][h]
                nc.sync.dma_start(gl[:].rearrange("(t p) f -> p t f", p=P), gs[:])
                nc.gpsimd.collective_compute(
                    "AllGather",
                    mybir.AluOpType.bypass,
                    replica_groups=[list(range(N_CORES))],
                    ins=[gl.opt()],
                    outs=[g_full[layer][h].opt()],
                )

            def layer_agg(layer, xout, bias):
                for t in range(T):
                    nch = [int(sched[t, 0]), int(sched[t, 1])]
                    tot = nch[0] + nch[1]
                    msg = msgp.tile([P, maxch, P], f32, tag="msg", name="msg")
                    s_sb = smatp.tile([P, maxch * P], f32, tag="smat", name="ssb")
                    co = int(chunk_off[t * 2])
                    nc.sync.dma_start(
                        s_sb[:, 0 : tot * P], smat_in[:, co * P : (co + tot) * P]
                    )
                    jo = 0
                    for h in range(2):
                        K = nch[h] * P
                        coh = int(chunk_off[t * 2 + h])
                        nc.gpsimd.dma_gather(
                            msg[:, jo : jo + nch[h], :],
                            g_full[layer][h][:],
                            idx_sb[:, coh * 8 : coh * 8 + nch[h] * 8],
                            K, K, P,
                            single_packet=False,
                        )
                        jo += nch[h]
                    diag = diagp.tile([P, P], f32, tag="diag", name="diag")
                    nc.vector.tensor_scalar(
                        diag[:], ident[:], dinv[:, t : t + 1], None,
                        mybir.AluOpType.mult,
                    )
                    ps = psagg.tile([P, P], f32, tag="ps_agg", name="psagg")
                    for j in range(tot):
                        nc.tensor.matmul(
                            ps[:], lhsT=msg[:, j, :],
                            rhs=s_sb[:, bass.ts(j, P)],
                            start=(j == 0), stop=False,
                        )
                    hh, ii = loc_tile(t)
                    nc.tensor.matmul(
                        ps[:], lhsT=gstage[hh][:, bass.ts(ii, P)], rhs=diag[:],
                        start=(tot == 0), stop=True,
                    )
                    nc.scalar.activation(
                        xout[hh][:, bass.ts(ii, P)], ps[:],
                        mybir.ActivationFunctionType.Relu, bias=bias[:],
                    )

            xT_stages = [xT[:, 0:BLKA], xT[:, BLKA:NPAD]]

            class _W:  # tiny adapter so phase_g_block can slice uniformly
                def __init__(self, aps):
                    self.aps = aps
                def __getitem__(self, h):
                    return self.aps[h]

            phase_g_block(_W(xT_stages), w1, 0, 0)
            phase_g_block(_W(xT_stages), w1, 0, 1)
            layer_agg(0, x1T, b1)
            phase_g_block(x1T, w2, 1, 0)
            phase_g_block(x1T, w2, 1, 1)
            layer_agg(1, x2T, b2)

            for t in range(T):
                hh, ii = loc_tile(t)
                ps = psphase.tile([P, P], f32, tag="ps_phase", name="psph")
                nc.tensor.matmul(
                    ps[:], lhsT=x1T[hh][:, bass.ts(ii, P)], rhs=lin1[:],
                    start=True, stop=False,
                )
                nc.tensor.matmul(
                    ps[:], lhsT=x2T[hh][:, bass.ts(ii, P)], rhs=lin2[:],
                    start=False, stop=True,
                )
                ot = otilep.tile([P, P], f32, tag="otile", name="otile")
                nc.vector.tensor_tensor(
                    out=ot[:], in0=ps[:], in1=linb[:], op=mybir.AluOpType.add
                )
                nc.sync.dma_start(out_v[:, t, :], ot[:])

    nc.compile()
    return nc


def kernel(x, edge_index, W1, b1, W2, b2, lin_W, lin_b):
    x = np.asarray(x, np.float32)
    edge_index = np.asarray(edge_index)
    W1 = np.asarray(W1, np.float32)
    W2 = np.asarray(W2, np.float32)
    b1 = np.asarray(b1, np.float32)
    b2 = np.asarray(b2, np.float32)
    lin_W = np.asarray(lin_W, np.float32)
    lin_b = np.asarray(lin_b, np.float32)

    plan, per_core = _preprocess(x, edge_index)
    nc = _build(plan)

    N, D, C, NPC = plan["N"], plan["D"], plan["C"], plan["NPC"]
    in_maps = []
    for c in range(C):
        pc = per_core[c]
        in_maps.append(
            {
                "xT": pc["xT"],
                "w1": W1,
                "w2": W2,
                "lin1": np.ascontiguousarray(lin_W[:D]),
                "lin2": np.ascontiguousarray(lin_W[D:]),
                "b1": b1[:, None].astype(np.float32),
                "b2": b2[:, None].astype(np.float32),
                "linb": np.tile(lin_b, (P, 1)).astype(np.float32),
                "dinv": pc["dinv"],
                "idx": pc["idx"],
                "smat": pc["smat"],
            }
        )

    last_err = None
    for _attempt in range(3):
        try:
            res = run_bass_kernel_spmd(nc, in_maps, list(range(C)))
            break
        except Exception as e:  # transient NRT device wedges happen
            last_err = e
    else:
        raise last_err

    out = np.empty((N, D), np.float32)
    for c in range(C):
        out[c * NPC : (c + 1) * NPC] = res.results[c]["out"][:NPC]
    return out


# revision 6
# speedup vs baseline: 1.0414x; 1.0300x over previous
"""2-layer GCN + JumpingKnowledge(cat) + Linear on 8 Trainium2 NeuronCores.

Strategy (graph-parallel, nodes sharded 6250/core):
  - g = dinv * (x @ W) computed per-core (TensorE + DVE), written to HBM,
    AllGather'd so every core holds the full node-feature table.
  - Message passing per destination tile (128 dsts): dma_gather pulls the
    unique source rows (fp32, 512B each) from the table; aggregation is a
    chain of TensorE matmuls  psum[feat,dst] += M_chunk^T @ S_chunk  where
    S (host-built, fp32) carries the symmetric-norm coefficients dinv[dst]
    (multiplicity-summed, dedup'd sources).  Self-loops use the local g
    tile against diag(dinv) - no gather needed.
  - relu(+bias) on ScalarE writes the transposed activations x^T directly,
    which feeds the next layer's matmuls without any transposes.
  - Final: out = x1 @ lin_W[:D] + x2 @ lin_W[D:] (+ lin_b) per tile.

dma_gather needs int16 indices, so the global table is built as two block
tables (each core contributes rows [0,3200) to table A and [3200,6272) to
table B).  Each block is AllGather'd separately so the collectives overlap
the gather stream of the previous work (the single Q7 descriptor-generation
core is the critical path at ~8.4ns/row; everything else hides under it).
"""
import numpy as np

import concourse.bass as bass
import concourse.bacc as bacc
import concourse.mybir as mybir
import concourse.tile as tile
from concourse._compat import get_trn_type
from concourse.bass_utils import run_bass_kernel_spmd
from concourse.library_config import mlp
from concourse.masks import make_identity

P = 128
N_CORES = 8

f32 = mybir.dt.float32
i16 = mybir.dt.int16


def _preprocess(x, edge_index):
    """Host-side (numpy): shard, block-split, dedup, build gather indices
    and the norm-coefficient matrices S."""
    N, D = x.shape
    assert D == P
    E = edge_index.shape[1]
    C = N_CORES
    NPC = (N + C - 1) // C
    assert NPC * C == N, "node count must split evenly across cores"
    NPAD = ((NPC + P - 1) // P) * P
    T = NPAD // P
    TA = (T + 1) // 2          # tiles in block A
    TB = T - TA
    BLKA, BLKB = TA * P, TB * P
    FULLA, FULLB = C * BLKA, C * BLKB
    assert FULLA <= 32768 and FULLB <= 32768

    src = edge_index[0].astype(np.int64)
    dst = edge_index[1].astype(np.int64)

    deg = np.bincount(dst, minlength=N).astype(np.float32) + 1.0
    dinv = (1.0 / np.sqrt(deg)).astype(np.float32)

    off = src % NPC
    blk = (off >= BLKA).astype(np.int64)
    row_in_blk = np.where(blk == 0, (src // NPC) * BLKA + off,
                          (src // NPC) * BLKB + (off - BLKA))
    core = dst // NPC
    dloc = dst % NPC
    tl = dloc // P
    dcol = dloc % P
    vals = dinv[dst]

    gkey = (core * T + tl) * 2 + blk
    order = np.lexsort((row_in_blk, gkey))
    gkey_s = gkey[order]
    rows_s = row_in_blk[order]
    dcol_s = dcol[order]
    vals_s = vals[order]
    n_groups = C * T * 2
    bounds = np.searchsorted(gkey_s, np.arange(n_groups + 1))

    uniq = [None] * n_groups
    invs = [None] * n_groups
    for g in range(n_groups):
        lo, hi = bounds[g], bounds[g + 1]
        r = rows_s[lo:hi]
        u = np.unique(r)
        uniq[g] = u
        invs[g] = np.searchsorted(u, r)

    # SPMD-uniform chunk schedule: max over cores per (tile, half)
    ch = np.zeros((C, T, 2), np.int64)
    for g in range(n_groups):
        c, rem = divmod(g, T * 2)
        t, h = divmod(rem, 2)
        ch[c, t, h] = (len(uniq[g]) + P - 1) // P
    sched = np.maximum(ch.max(axis=0), 1)  # [T, 2]
    SCH = int(sched.sum())

    flat_sched = sched.reshape(-1)
    chunk_off = np.zeros(T * 2 + 1, np.int64)
    np.cumsum(flat_sched, out=chunk_off[1:])

    per_core = []
    for c in range(C):
        idx_cols = np.zeros((P, SCH * 8), np.int16)
        cap = E // C + 4 * SCH * P + E // 16
        flat_rows = np.zeros(cap, np.int64)
        flat_dcol = np.zeros(cap, np.int64)
        flat_vals = np.zeros(cap, np.float32)
        n_e = 0
        for t in range(T):
            for h in range(2):
                g = (c * T + t) * 2 + h
                nch = int(sched[t, h])
                K = nch * P
                u = uniq[g]
                u_pad = np.zeros(K, np.int64)
                u_pad[: len(u)] = u
                wrapped = u_pad.astype(np.int16).reshape(nch * 8, 16).T
                co = int(chunk_off[t * 2 + h])
                idx_cols[:, co * 8 : (co + nch) * 8] = np.tile(wrapped, (8, 1))
                lo, hi = bounds[g], bounds[g + 1]
                ne = hi - lo
                flat_rows[n_e : n_e + ne] = co * P + invs[g]
                flat_dcol[n_e : n_e + ne] = dcol_s[lo:hi]
                flat_vals[n_e : n_e + ne] = vals_s[lo:hi]
                n_e += ne
        flat = flat_rows[:n_e] * P + flat_dcol[:n_e]
        s_core = np.bincount(flat, weights=flat_vals[:n_e], minlength=SCH * P * P)
        s_core = s_core.astype(np.float32).reshape(SCH, P, P)
        smat = np.ascontiguousarray(s_core.transpose(1, 0, 2)).reshape(P, SCH * P)

        xT = np.zeros((P, NPAD), np.float32)
        xT[:, :NPC] = x[c * NPC : (c + 1) * NPC].T
        dv = np.zeros(NPAD, np.float32)
        dv[:NPC] = dinv[c * NPC : (c + 1) * NPC]
        dinv_tiles = np.ascontiguousarray(dv.reshape(T, P).T)  # [P, T]

        per_core.append({"xT": xT, "dinv": dinv_tiles, "idx": idx_cols, "smat": smat})

    plan = {
        "N": N, "D": D, "E": E, "C": C, "NPC": NPC, "NPAD": NPAD, "T": T,
        "TA": TA, "TB": TB, "BLKA": BLKA, "BLKB": BLKB,
        "FULLA": FULLA, "FULLB": FULLB, "SCH": SCH,
        "sched": sched, "chunk_off": chunk_off,
    }
    return plan, per_core


def _build(plan):
    T, TA, TB = plan["T"], plan["TA"], plan["TB"]
    NPAD = plan["NPAD"]
    BLKA, BLKB = plan["BLKA"], plan["BLKB"]
    FULLA, FULLB = plan["FULLA"], plan["FULLB"]
    SCH = plan["SCH"]
    sched = plan["sched"]
    chunk_off = plan["chunk_off"]
    maxch = int(sched.sum(axis=1).max())

    nc = bacc.Bacc(
        get_trn_type() or "TRN2",
        target_bir_lowering=False,
        debug=False,
        num_devices=N_CORES,
    )
    xT_in = nc.dram_tensor("xT", [P, NPAD], f32, kind="ExternalInput").ap()
    w1_in = nc.dram_tensor("w1", [P, P], f32, kind="ExternalInput").ap()
    w2_in = nc.dram_tensor("w2", [P, P], f32, kind="ExternalInput").ap()
    lin1_in = nc.dram_tensor("lin1", [P, P], f32, kind="ExternalInput").ap()
    lin2_in = nc.dram_tensor("lin2", [P, P], f32, kind="ExternalInput").ap()
    b1_in = nc.dram_tensor("b1", [P, 1], f32, kind="ExternalInput").ap()
    b2_in = nc.dram_tensor("b2", [P, 1], f32, kind="ExternalInput").ap()
    linb_in = nc.dram_tensor("linb", [P, P], f32, kind="ExternalInput").ap()
    dinv_in = nc.dram_tensor("dinv", [P, T], f32, kind="ExternalInput").ap()
    idx_in = nc.dram_tensor("idx", [P, SCH * 8], i16, kind="ExternalInput").ap()
    smat_in = nc.dram_tensor("smat", [P, SCH * P], f32, kind="ExternalInput").ap()
    out_ap = nc.dram_tensor("out", [NPAD, P], f32, kind="ExternalOutput").ap()
    out_v = out_ap.rearrange("(t p) f -> p t f", p=P)

    nc.gpsimd.load_library(mlp)

    with tile.TileContext(nc) as tc:
        with (
            tc.tile_pool(name="dram", bufs=1, space="DRAM") as dram,
            tc.tile_pool(name="consts", bufs=1) as consts,
            tc.tile_pool(name="xTp", bufs=1) as xTp,
            tc.tile_pool(name="stages", bufs=1) as stages,
            tc.tile_pool(name="msg", bufs=3) as msgp,
            tc.tile_pool(name="smat", bufs=3) as smatp,
            tc.tile_pool(name="diag", bufs=3) as diagp,
            tc.tile_pool(name="otile", bufs=3) as otilep,
            tc.tile_pool(name="ps_phase", bufs=4, space="PSUM") as psphase,
            tc.tile_pool(name="ps_agg", bufs=4, space="PSUM") as psagg,
        ):
            xT = xTp.tile([P, NPAD], f32)
            nc.sync.dma_start(xT[:], xT_in[:])
            w1 = consts.tile([P, P], f32, tag="w1")
            nc.sync.dma_start(w1[:], w1_in[:])
            w2 = consts.tile([P, P], f32, tag="w2")
            nc.sync.dma_start(w2[:], w2_in[:])
            lin1 = consts.tile([P, P], f32, tag="lin1")
            nc.sync.dma_start(lin1[:], lin1_in[:])
            lin2 = consts.tile([P, P], f32, tag="lin2")
            nc.sync.dma_start(lin2[:], lin2_in[:])
            b1 = consts.tile([P, 1], f32, tag="b1")
            nc.sync.dma_start(b1[:], b1_in[:])
            b2 = consts.tile([P, 1], f32, tag="b2")
            nc.sync.dma_start(b2[:], b2_in[:])
            linb = consts.tile([P, P], f32, tag="linb")
            nc.sync.dma_start(linb[:], linb_in[:])
            dinv = consts.tile([P, T], f32, tag="dinv")
            nc.sync.dma_start(dinv[:], dinv_in[:])
            idx_sb = consts.tile([P, SCH * 8], i16, tag="idx")
            nc.sync.dma_start(idx_sb[:], idx_in[:])
            ident = consts.tile([P, P], f32, tag="ident")
            make_identity(nc, ident[:])

            # per-block stage tiles: [P, BLKA] and [P, BLKB]
            def blk_pair(tag):
                a = stages.tile([P, BLKA], f32, tag=f"{tag}A", name=f"{tag}A")
                b = stages.tile([P, BLKB], f32, tag=f"{tag}B", name=f"{tag}B")
                return [a, b]

            gstage = blk_pair("gstage")
            x1T = blk_pair("x1T")
            x2T = blk_pair("x2T")

            g_loc = [[None, None], [None, None]]
            g_full = [[None, None], [None, None]]
            for layer in range(2):
                for h, (blkrows, fullrows) in enumerate([(BLKA, FULLA), (BLKB, FULLB)]):
                    g_loc[layer][h] = dram.tile(
                        [blkrows, P], f32, tag=f"gloc{layer}{h}", name=f"gloc{layer}{h}"
                    )
                    g_full[layer][h] = dram.tile(
                        [fullrows, P], f32, tag=f"gfull{layer}{h}", name=f"gfull{layer}{h}"
                    )

            def loc_tile(t):
                """(stage-half h, column-tile index within that half)"""
                return (0, t) if t < TA else (1, t - TA)

            def phase_g_block(src_stages, w_tile, layer, h):
                t0 = 0 if h == 0 else TA
                nt = TA if h == 0 else TB
                gs = gstage[h]
                for i in range(nt):
                    t = t0 + i
                    hh, ii = loc_tile(t)
                    ps = psphase.tile([P, P], f32, tag="ps_phase", name="psph")
                    nc.tensor.matmul(
                        ps[:], lhsT=src_stages[hh][:, bass.ts(ii, P)], rhs=w_tile[:],
                        start=True, stop=True,
                    )
                    nc.vector.tensor_scalar(
                        gs[:, bass.ts(i, P)], ps[:],
                        dinv[:, t : t + 1], None, mybir.AluOpType.mult,
                    )
                gl = g_loc[layer][h]
                nc.sync.dma_start(gl[:].rearrange("(t p) f -> p t f", p=P), gs[:])
                nc.gpsimd.collective_compute(
                    "AllGather",
                    mybir.AluOpType.bypass,
                    replica_groups=[list(range(N_CORES))],
                    ins=[gl.opt()],
                    outs=[g_full[layer][h].opt()],
                )

            def layer_agg(layer, xout, bias, hooks=None):
                for t in range(T):
                    if hooks and t in hooks:
                        hooks[t]()
                    nch = [int(sched[t, 0]), int(sched[t, 1])]
                    tot = nch[0] + nch[1]
                    msg = msgp.tile([P, maxch, P], f32, tag="msg", name="msg")
                    s_sb = smatp.tile([P, maxch * P], f32, tag="smat", name="ssb")
                    co = int(chunk_off[t * 2])
                    nc.sync.dma_start(
                        s_sb[:, 0 : tot * P], smat_in[:, co * P : (co + tot) * P]
                    )
                    jo = 0
                    for h in range(2):
                        K = nch[h] * P
                        coh = int(chunk_off[t * 2 + h])
                        nc.gpsimd.dma_gather(
                            msg[:, jo : jo + nch[h], :],
                            g_full[layer][h][:],
                            idx_sb[:, coh * 8 : coh * 8 + nch[h] * 8],
                            K, K, P,
                            single_packet=False,
                        )
                        jo += nch[h]
                    diag = diagp.tile([P, P], f32, tag="diag", name="diag")
                    nc.vector.tensor_scalar(
                        diag[:], ident[:], dinv[:, t : t + 1], None,
                        mybir.AluOpType.mult,
                    )
                    ps = psagg.tile([P, P], f32, tag="ps_agg", name="psagg")
                    for j in range(tot):
                        nc.tensor.matmul(
                            ps[:], lhsT=msg[:, j, :],
                            rhs=s_sb[:, bass.ts(j, P)],
                            start=(j == 0), stop=False,
                        )
                    hh, ii = loc_tile(t)
                    nc.tensor.matmul(
                        ps[:], lhsT=gstage[hh][:, bass.ts(ii, P)], rhs=diag[:],
                        start=(tot == 0), stop=True,
                    )
                    nc.scalar.activation(
                        xout[hh][:, bass.ts(ii, P)], ps[:],
                        mybir.ActivationFunctionType.Relu, bias=bias[:],
                    )

            xT_stages = [xT[:, 0:BLKA], xT[:, BLKA:NPAD]]

            class _W:  # tiny adapter so phase_g_block can slice uniformly
                def __init__(self, aps):
                    self.aps = aps
                def __getitem__(self, h):
                    return self.aps[h]

            phase_g_block(_W(xT_stages), w1, 0, 0)
            phase_g_block(_W(xT_stages), w1, 0, 1)
            # layer-2 block-A table (x1T-A ready after L1 tile TA-1) is
            # AllGather'd mid-stream: its gpsimd trigger is emitted between
            # L1 gathers so the collective overlaps the gather stream.
            layer_agg(
                0, x1T, b1,
                hooks={TA + 2: lambda: phase_g_block(x1T, w2, 1, 0)},
            )
            phase_g_block(x1T, w2, 1, 1)
            layer_agg(1, x2T, b2)

            for t in range(T):
                hh, ii = loc_tile(t)
                ps = psphase.tile([P, P], f32, tag="ps_phase", name="psph")
                nc.tensor.matmul(
                    ps[:], lhsT=x1T[hh][:, bass.ts(ii, P)], rhs=lin1[:],
                    start=True, stop=False,
                )
                nc.tensor.matmul(
                    ps[:], lhsT=x2T[hh][:, bass.ts(ii, P)], rhs=lin2[:],
                    start=False, stop=True,
                )
                ot = otilep.tile([P, P], f32, tag="otile", name="otile")
                nc.vector.tensor_tensor(
                    out=ot[:], in0=ps[:], in1=linb[:], op=mybir.AluOpType.add
                )
                nc.sync.dma_start(out_v[:, t, :], ot[:])

    nc.compile()
    return nc


def kernel(x, edge_index, W1, b1, W2, b2, lin_W, lin_b):
    x = np.asarray(x, np.float32)
    edge_index = np.asarray(edge_index)
    W1 = np.asarray(W1, np.float32)
    W2 = np.asarray(W2, np.float32)
    b1 = np.asarray(b1, np.float32)
    b2 = np.asarray(b2, np.float32)
    lin_W = np.asarray(lin_W, np.float32)
    lin_b = np.asarray(lin_b, np.float32)

    plan, per_core = _preprocess(x, edge_index)
    nc = _build(plan)

    N, D, C, NPC = plan["N"], plan["D"], plan["C"], plan["NPC"]
    in_maps = []
    for c in range(C):
        pc = per_core[c]
        in_maps.append(
            {
                "xT": pc["xT"],
                "w1": W1,
                "w2": W2,
                "lin1": np.ascontiguousarray(lin_W[:D]),
                "lin2": np.ascontiguousarray(lin_W[D:]),
                "b1": b1[:, None].astype(np.float32),
                "b2": b2[:, None].astype(np.float32),
                "linb": np.tile(lin_b, (P, 1)).astype(np.float32),
                "dinv": pc["dinv"],
                "idx": pc["idx"],
                "smat": pc["smat"],
            }
        )

    last_err = None
    for _attempt in range(3):
        try:
            res = run_bass_kernel_spmd(nc, in_maps, list(range(C)))
            break
        except Exception as e:  # transient NRT device wedges happen
            last_err = e
    else:
        raise last_err

    out = np.empty((N, D), np.float32)
    for c in range(C):
        out[c * NPC : (c + 1) * NPC] = res.results[c]["out"][:NPC]
    return out


# revision 9
# speedup vs baseline: 1.0670x; 1.0246x over previous
"""2-layer GCN + JumpingKnowledge(cat) + Linear on 8 Trainium2 NeuronCores.

Strategy (graph-parallel, nodes sharded 6250/core):
  - g = dinv * (x @ W) computed per-core (TensorE + DVE), written to HBM,
    AllGather'd so every core holds the full node-feature table.
  - Message passing per destination tile (128 dsts): dma_gather pulls the
    unique source rows (fp32, 512B each) from the table; aggregation is a
    chain of TensorE matmuls  psum[feat,dst] += M_chunk^T @ S_chunk  where
    S (host-built, fp32) carries the symmetric-norm coefficients dinv[dst]
    (multiplicity-summed, dedup'd sources).  Self-loops use the local g
    tile against diag(dinv) - no gather needed.
  - relu(+bias) on ScalarE writes the transposed activations x^T directly,
    which feeds the next layer's matmuls without any transposes.
  - Final: out = x1 @ lin_W[:D] + x2 @ lin_W[D:] (+ lin_b) per tile.

dma_gather needs int16 indices, so the global table is built as two block
tables (each core contributes rows [0,3200) to table A and [3200,6272) to
table B).  Each block is AllGather'd separately so the collectives overlap
the gather stream of the previous work (the single Q7 descriptor-generation
core is the critical path at ~8.4ns/row; everything else hides under it).
"""
import numpy as np

import concourse.bass as bass
import concourse.bacc as bacc
import concourse.mybir as mybir
import concourse.tile as tile
from concourse._compat import get_trn_type
from concourse.bass_utils import run_bass_kernel_spmd
from concourse.library_config import mlp
from concourse.masks import make_identity

P = 128
N_CORES = 8

f32 = mybir.dt.float32
i16 = mybir.dt.int16


def _preprocess(x, edge_index):
    """Host-side (numpy): shard, block-split, dedup, build gather indices
    and the norm-coefficient matrices S."""
    N, D = x.shape
    assert D == P
    E = edge_index.shape[1]
    C = N_CORES
    NPC = (N + C - 1) // C
    assert NPC * C == N, "node count must split evenly across cores"
    NPAD = ((NPC + P - 1) // P) * P
    T = NPAD // P
    TA = (T + 1) // 2          # tiles in block A
    TB = T - TA
    BLKA, BLKB = TA * P, TB * P
    FULLA, FULLB = C * BLKA, C * BLKB
    assert FULLA <= 32768 and FULLB <= 32768

    src = edge_index[0].astype(np.int64)
    dst = edge_index[1].astype(np.int64)

    deg = np.bincount(dst, minlength=N).astype(np.float32) + 1.0
    dinv = (1.0 / np.sqrt(deg)).astype(np.float32)

    off = src % NPC
    blk = (off >= BLKA).astype(np.int64)
    row_in_blk = np.where(blk == 0, (src // NPC) * BLKA + off,
                          (src // NPC) * BLKB + (off - BLKA))
    core = dst // NPC
    dloc = dst % NPC
    tl = dloc // P
    dcol = dloc % P
    vals = dinv[dst]

    gkey = (core * T + tl) * 2 + blk
    order = np.lexsort((row_in_blk, gkey))
    gkey_s = gkey[order]
    rows_s = row_in_blk[order]
    dcol_s = dcol[order]
    vals_s = vals[order]
    n_groups = C * T * 2
    bounds = np.searchsorted(gkey_s, np.arange(n_groups + 1))

    uniq = [None] * n_groups
    invs = [None] * n_groups
    for g in range(n_groups):
        lo, hi = bounds[g], bounds[g + 1]
        r = rows_s[lo:hi]
        u = np.unique(r)
        uniq[g] = u
        invs[g] = np.searchsorted(u, r)

    # SPMD-uniform chunk schedule: max over cores per (tile, half)
    ch = np.zeros((C, T, 2), np.int64)
    for g in range(n_groups):
        c, rem = divmod(g, T * 2)
        t, h = divmod(rem, 2)
        ch[c, t, h] = (len(uniq[g]) + P - 1) // P
    sched = np.maximum(ch.max(axis=0), 1)  # [T, 2]
    SCH = int(sched.sum())

    flat_sched = sched.reshape(-1)
    chunk_off = np.zeros(T * 2 + 1, np.int64)
    np.cumsum(flat_sched, out=chunk_off[1:])

    per_core = []
    for c in range(C):
        idx_cols = np.zeros((P, SCH * 8), np.int16)
        cap = E // C + 4 * SCH * P + E // 16
        flat_rows = np.zeros(cap, np.int64)
        flat_dcol = np.zeros(cap, np.int64)
        flat_vals = np.zeros(cap, np.float32)
        n_e = 0
        for t in range(T):
            for h in range(2):
                g = (c * T + t) * 2 + h
                nch = int(sched[t, h])
                K = nch * P
                u = uniq[g]
                u_pad = np.zeros(K, np.int64)
                u_pad[: len(u)] = u
                wrapped = u_pad.astype(np.int16).reshape(nch * 8, 16).T
                co = int(chunk_off[t * 2 + h])
                idx_cols[:, co * 8 : (co + nch) * 8] = np.tile(wrapped, (8, 1))
                lo, hi = bounds[g], bounds[g + 1]
                ne = hi - lo
                flat_rows[n_e : n_e + ne] = co * P + invs[g]
                flat_dcol[n_e : n_e + ne] = dcol_s[lo:hi]
                flat_vals[n_e : n_e + ne] = vals_s[lo:hi]
                n_e += ne
        flat = flat_rows[:n_e] * P + flat_dcol[:n_e]
        s_core = np.bincount(flat, weights=flat_vals[:n_e], minlength=SCH * P * P)
        s_core = s_core.astype(np.float32).reshape(SCH, P, P)
        smat = np.ascontiguousarray(s_core.transpose(1, 0, 2)).reshape(P, SCH * P)

        xT = np.zeros((P, NPAD), np.float32)
        xT[:, :NPC] = x[c * NPC : (c + 1) * NPC].T
        dv = np.zeros(NPAD, np.float32)
        dv[:NPC] = dinv[c * NPC : (c + 1) * NPC]
        dinv_tiles = np.ascontiguousarray(dv.reshape(T, P).T)  # [P, T]

        per_core.append({"xT": xT, "dinv": dinv_tiles, "idx": idx_cols, "smat": smat})

    plan = {
        "N": N, "D": D, "E": E, "C": C, "NPC": NPC, "NPAD": NPAD, "T": T,
        "TA": TA, "TB": TB, "BLKA": BLKA, "BLKB": BLKB,
        "FULLA": FULLA, "FULLB": FULLB, "SCH": SCH,
        "sched": sched, "chunk_off": chunk_off,
    }
    return plan, per_core


def _build(plan):
    T, TA, TB = plan["T"], plan["TA"], plan["TB"]
    NPAD = plan["NPAD"]
    BLKA, BLKB = plan["BLKA"], plan["BLKB"]
    FULLA, FULLB = plan["FULLA"], plan["FULLB"]
    SCH = plan["SCH"]
    sched = plan["sched"]
    chunk_off = plan["chunk_off"]
    maxch = int(sched.sum(axis=1).max())

    nc = bacc.Bacc(
        get_trn_type() or "TRN2",
        target_bir_lowering=False,
        debug=False,
        num_devices=N_CORES,
    )
    xT_in = nc.dram_tensor("xT", [P, NPAD], f32, kind="ExternalInput").ap()
    w1_in = nc.dram_tensor("w1", [P, P], f32, kind="ExternalInput").ap()
    w2_in = nc.dram_tensor("w2", [P, P], f32, kind="ExternalInput").ap()
    lin1_in = nc.dram_tensor("lin1", [P, P], f32, kind="ExternalInput").ap()
    lin2_in = nc.dram_tensor("lin2", [P, P], f32, kind="ExternalInput").ap()
    b1_in = nc.dram_tensor("b1", [P, 1], f32, kind="ExternalInput").ap()
    b2_in = nc.dram_tensor("b2", [P, 1], f32, kind="ExternalInput").ap()
    linb_in = nc.dram_tensor("linb", [P, P], f32, kind="ExternalInput").ap()
    dinv_in = nc.dram_tensor("dinv", [P, T], f32, kind="ExternalInput").ap()
    idx_in = nc.dram_tensor("idx", [P, SCH * 8], i16, kind="ExternalInput").ap()
    smat_in = nc.dram_tensor("smat", [P, SCH * P], f32, kind="ExternalInput").ap()
    out_ap = nc.dram_tensor("out", [NPAD, P], f32, kind="ExternalOutput").ap()
    out_v = out_ap.rearrange("(t p) f -> p t f", p=P)

    nc.gpsimd.load_library(mlp)

    with tile.TileContext(nc) as tc:
        with (
            tc.tile_pool(name="dram", bufs=1, space="DRAM") as dram,
            tc.tile_pool(name="consts", bufs=1) as consts,
            tc.tile_pool(name="xTp", bufs=1) as xTp,
            tc.tile_pool(name="stages", bufs=1) as stages,
            tc.tile_pool(name="msg", bufs=3) as msgp,
            tc.tile_pool(name="smat", bufs=3) as smatp,
            tc.tile_pool(name="diag", bufs=3) as diagp,
            tc.tile_pool(name="otile", bufs=3) as otilep,
            tc.tile_pool(name="ps_phase", bufs=4, space="PSUM") as psphase,
            tc.tile_pool(name="ps_agg", bufs=4, space="PSUM") as psagg,
        ):
            xT = xTp.tile([P, NPAD], f32)
            nc.sync.dma_start(xT[:], xT_in[:])
            w1 = consts.tile([P, P], f32, tag="w1")
            nc.sync.dma_start(w1[:], w1_in[:])
            w2 = consts.tile([P, P], f32, tag="w2")
            nc.sync.dma_start(w2[:], w2_in[:])
            lin1 = consts.tile([P, P], f32, tag="lin1")
            nc.sync.dma_start(lin1[:], lin1_in[:])
            lin2 = consts.tile([P, P], f32, tag="lin2")
            nc.sync.dma_start(lin2[:], lin2_in[:])
            b1 = consts.tile([P, 1], f32, tag="b1")
            nc.sync.dma_start(b1[:], b1_in[:])
            b2 = consts.tile([P, 1], f32, tag="b2")
            nc.sync.dma_start(b2[:], b2_in[:])
            linb = consts.tile([P, P], f32, tag="linb")
            nc.sync.dma_start(linb[:], linb_in[:])
            dinv = consts.tile([P, T], f32, tag="dinv")
            nc.sync.dma_start(dinv[:], dinv_in[:])
            idx_sb = consts.tile([P, SCH * 8], i16, tag="idx")
            nc.sync.dma_start(idx_sb[:], idx_in[:])
            ident = consts.tile([P, P], f32, tag="ident")
            make_identity(nc, ident[:])

            # per-block stage tiles: [P, BLKA] and [P, BLKB]
            def blk_pair(tag):
                a = stages.tile([P, BLKA], f32, tag=f"{tag}A", name=f"{tag}A")
                b = stages.tile([P, BLKB], f32, tag=f"{tag}B", name=f"{tag}B")
                return [a, b]

            gstage = blk_pair("gstage")
            x1T = blk_pair("x1T")
            x2T = blk_pair("x2T")

            g_loc = [[None, None], [None, None]]
            g_full = [[None, None], [None, None]]
            for layer in range(2):
                for h, (blkrows, fullrows) in enumerate([(BLKA, FULLA), (BLKB, FULLB)]):
                    g_loc[layer][h] = dram.tile(
                        [blkrows, P], f32, tag=f"gloc{layer}{h}", name=f"gloc{layer}{h}"
                    )
                    g_full[layer][h] = dram.tile(
                        [fullrows, P], f32, tag=f"gfull{layer}{h}", name=f"gfull{layer}{h}"
                    )

            def loc_tile(t):
                """(stage-half h, column-tile index within that half)"""
                return (0, t) if t < TA else (1, t - TA)

            def phase_g_block(src_stages, w_tile, layer, h):
                t0 = 0 if h == 0 else TA
                nt = TA if h == 0 else TB
                gs = gstage[h]
                for i in range(nt):
                    t = t0 + i
                    hh, ii = loc_tile(t)
                    ps = psphase.tile([P, P], f32, tag="ps_phase", name="psph")
                    nc.tensor.matmul(
                        ps[:], lhsT=src_stages[hh][:, bass.ts(ii, P)], rhs=w_tile[:],
                        start=True, stop=True,
                    )
                    nc.vector.tensor_scalar(
                        gs[:, bass.ts(i, P)], ps[:],
                        dinv[:, t : t + 1], None, mybir.AluOpType.mult,
                    )
                gl = g_loc[layer][h]
                nc.sync.dma_start(gl[:].rearrange("(t p) f -> p t f", p=P), gs[:])
                nc.gpsimd.collective_compute(
                    "AllGather",
                    mybir.AluOpType.bypass,
                    replica_groups=[list(range(N_CORES))],
                    ins=[gl.opt()],
                    outs=[g_full[layer][h].opt()],
                )

            partial = stages.tile([P, NPAD], f32, tag="partial", name="partial")

            def layer_agg(layer, xout, bias, hooks=None):
                # pass 1: block-A chunks only -> partial (so the stream only
                # needs table A, which is AllGather'd first)
                for t in range(T):
                    c0 = int(sched[t, 0])
                    K = c0 * P
                    co = int(chunk_off[t * 2])
                    msg = msgp.tile([P, maxch, P], f32, tag="msg", name="msg")
                    s_sb = smatp.tile([P, maxch * P], f32, tag="smat", name="ssb")
                    nc.sync.dma_start(
                        s_sb[:, 0 : c0 * P], smat_in[:, co * P : (co + c0) * P]
                    )
                    nc.gpsimd.dma_gather(
                        msg[:, 0:c0, :],
                        g_full[layer][0][:],
                        idx_sb[:, co * 8 : (co + c0) * 8],
                        K, K, P,
                        single_packet=False,
                    )
                    ps = psagg.tile([P, P], f32, tag="ps_agg", name="psagg")
                    for j in range(c0):
                        nc.tensor.matmul(
                            ps[:], lhsT=msg[:, j, :],
                            rhs=s_sb[:, bass.ts(j, P)],
                            start=(j == 0), stop=(j == c0 - 1),
                        )
                    nc.vector.tensor_copy(out=partial[:, bass.ts(t, P)], in_=ps[:])
                # pass 2: block-B chunks + self-loop, add partial, relu
                for t in range(T):
                    if hooks and t in hooks:
                        hooks[t]()
                    c1 = int(sched[t, 1])
                    K = c1 * P
                    co = int(chunk_off[t * 2 + 1])
                    msg = msgp.tile([P, maxch, P], f32, tag="msg", name="msg")
                    s_sb = smatp.tile([P, maxch * P], f32, tag="smat", name="ssb")
                    nc.sync.dma_start(
                        s_sb[:, 0 : c1 * P], smat_in[:, co * P : (co + c1) * P]
                    )
                    nc.gpsimd.dma_gather(
                        msg[:, 0:c1, :],
                        g_full[layer][1][:],
                        idx_sb[:, co * 8 : (co + c1) * 8],
                        K, K, P,
                        single_packet=False,
                    )
                    diag = diagp.tile([P, P], f32, tag="diag", name="diag")
                    nc.vector.tensor_scalar(
                        diag[:], ident[:], dinv[:, t : t + 1], None,
                        mybir.AluOpType.mult,
                    )
                    ps = psagg.tile([P, P], f32, tag="ps_agg", name="psagg")
                    for j in range(c1):
                        nc.tensor.matmul(
                            ps[:], lhsT=msg[:, j, :],
                            rhs=s_sb[:, bass.ts(j, P)],
                            start=(j == 0), stop=False,
                        )
                    hh, ii = loc_tile(t)
                    nc.tensor.matmul(
                        ps[:], lhsT=gstage[hh][:, bass.ts(ii, P)], rhs=diag[:],
                        start=(c1 == 0), stop=True,
                    )
                    nc.vector.tensor_tensor(
                        out=ps[:], in0=ps[:], in1=partial[:, bass.ts(t, P)],
                        op=mybir.AluOpType.add,
                    )
                    nc.scalar.activation(
                        xout[hh][:, bass.ts(ii, P)], ps[:],
                        mybir.ActivationFunctionType.Relu, bias=bias[:],
                    )

            xT_stages = [xT[:, 0:BLKA], xT[:, BLKA:NPAD]]

            class _W:  # tiny adapter so phase_g_block can slice uniformly
                def __init__(self, aps):
                    self.aps = aps
                def __getitem__(self, h):
                    return self.aps[h]

            phase_g_block(_W(xT_stages), w1, 0, 0)
            phase_g_block(_W(xT_stages), w1, 0, 1)
            # layer-2 block-A table (x1T-A ready after L1 tile TA-1) is
            # AllGather'd mid-stream: its gpsimd trigger is emitted between
            # L1 gathers so the collective overlaps the gather stream.
            layer_agg(
                0, x1T, b1,
                hooks={TA + 2: lambda: phase_g_block(x1T, w2, 1, 0)},
            )
            phase_g_block(x1T, w2, 1, 1)
            layer_agg(1, x2T, b2)

            for t in range(T):
                hh, ii = loc_tile(t)
                ps = psphase.tile([P, P], f32, tag="ps_phase", name="psph")
                nc.tensor.matmul(
                    ps[:], lhsT=x1T[hh][:, bass.ts(ii, P)], rhs=lin1[:],
                    start=True, stop=False,
                )
                nc.tensor.matmul(
                    ps[:], lhsT=x2T[hh][:, bass.ts(ii, P)], rhs=lin2[:],
                    start=False, stop=True,
                )
                ot = otilep.tile([P, P], f32, tag="otile", name="otile")
                nc.vector.tensor_tensor(
                    out=ot[:], in0=ps[:], in1=linb[:], op=mybir.AluOpType.add
                )
                nc.sync.dma_start(out_v[:, t, :], ot[:])

    nc.compile()
    return nc


def kernel(x, edge_index, W1, b1, W2, b2, lin_W, lin_b):
    x = np.asarray(x, np.float32)
    edge_index = np.asarray(edge_index)
    W1 = np.asarray(W1, np.float32)
    W2 = np.asarray(W2, np.float32)
    b1 = np.asarray(b1, np.float32)
    b2 = np.asarray(b2, np.float32)
    lin_W = np.asarray(lin_W, np.float32)
    lin_b = np.asarray(lin_b, np.float32)

    plan, per_core = _preprocess(x, edge_index)
    nc = _build(plan)

    N, D, C, NPC = plan["N"], plan["D"], plan["C"], plan["NPC"]
    in_maps = []
    for c in range(C):
        pc = per_core[c]
        in_maps.append(
            {
                "xT": pc["xT"],
                "w1": W1,
                "w2": W2,
                "lin1": np.ascontiguousarray(lin_W[:D]),
                "lin2": np.ascontiguousarray(lin_W[D:]),
                "b1": b1[:, None].astype(np.float32),
                "b2": b2[:, None].astype(np.float32),
                "linb": np.tile(lin_b, (P, 1)).astype(np.float32),
                "dinv": pc["dinv"],
                "idx": pc["idx"],
                "smat": pc["smat"],
            }
        )

    last_err = None
    for _attempt in range(3):
        try:
            res = run_bass_kernel_spmd(nc, in_maps, list(range(C)))
            break
        except Exception as e:  # transient NRT device wedges happen
            last_err = e
    else:
        raise last_err

    out = np.empty((N, D), np.float32)
    for c in range(C):
        out[c * NPC : (c + 1) * NPC] = res.results[c]["out"][:NPC]
    return out


# revision 10
# speedup vs baseline: 1.0947x; 1.0259x over previous
"""2-layer GCN + JumpingKnowledge(cat) + Linear on 8 Trainium2 NeuronCores.

Strategy (graph-parallel, nodes sharded 6250/core):
  - g = dinv * (x @ W) computed per-core (TensorE + DVE), written to HBM,
    AllGather'd so every core holds the full node-feature table.
  - Message passing per destination tile (128 dsts): dma_gather pulls the
    unique source rows (fp32, 512B each) from the table; aggregation is a
    chain of TensorE matmuls  psum[feat,dst] += M_chunk^T @ S_chunk  where
    S (host-built, fp32) carries the symmetric-norm coefficients dinv[dst]
    (multiplicity-summed, dedup'd sources).  Self-loops use the local g
    tile against diag(dinv) - no gather needed.
  - relu(+bias) on ScalarE writes the transposed activations x^T directly,
    which feeds the next layer's matmuls without any transposes.
  - Final: out = x1 @ lin_W[:D] + x2 @ lin_W[D:] (+ lin_b) per tile.

dma_gather needs int16 indices, so the global table is built as two block
tables (each core contributes rows [0,3200) to table A and [3200,6272) to
table B).  Each block is AllGather'd separately so the collectives overlap
the gather stream of the previous work (the single Q7 descriptor-generation
core is the critical path at ~8.4ns/row; everything else hides under it).
"""
import numpy as np

import concourse.bass as bass
import concourse.bacc as bacc
import concourse.mybir as mybir
import concourse.tile as tile
from concourse._compat import get_trn_type
from concourse.bass_utils import run_bass_kernel_spmd
from concourse.library_config import mlp
from concourse.masks import make_identity

P = 128
N_CORES = 8

f32 = mybir.dt.float32
i16 = mybir.dt.int16


def _preprocess(x, edge_index):
    """Host-side (numpy): shard, block-split, dedup, build gather indices
    and the norm-coefficient matrices S."""
    N, D = x.shape
    assert D == P
    E = edge_index.shape[1]
    C = N_CORES
    NPC = (N + C - 1) // C
    assert NPC * C == N, "node count must split evenly across cores"
    NPAD = ((NPC + P - 1) // P) * P
    T = NPAD // P
    TA = (T + 1) // 2          # tiles in block A
    TB = T - TA
    BLKA, BLKB = TA * P, TB * P
    FULLA, FULLB = C * BLKA, C * BLKB
    assert FULLA <= 32768 and FULLB <= 32768

    src = edge_index[0].astype(np.int64)
    dst = edge_index[1].astype(np.int64)

    deg = np.bincount(dst, minlength=N).astype(np.float32) + 1.0
    dinv = (1.0 / np.sqrt(deg)).astype(np.float32)

    off = src % NPC
    blk = (off >= BLKA).astype(np.int64)
    row_in_blk = np.where(blk == 0, (src // NPC) * BLKA + off,
                          (src // NPC) * BLKB + (off - BLKA))
    core = dst // NPC
    dloc = dst % NPC
    tl = dloc // P
    dcol = dloc % P
    vals = dinv[dst]

    gkey = (core * T + tl) * 2 + blk
    order = np.lexsort((row_in_blk, gkey))
    gkey_s = gkey[order]
    rows_s = row_in_blk[order]
    dcol_s = dcol[order]
    vals_s = vals[order]
    n_groups = C * T * 2
    bounds = np.searchsorted(gkey_s, np.arange(n_groups + 1))

    uniq = [None] * n_groups
    invs = [None] * n_groups
    for g in range(n_groups):
        lo, hi = bounds[g], bounds[g + 1]
        r = rows_s[lo:hi]
        u = np.unique(r)
        uniq[g] = u
        invs[g] = np.searchsorted(u, r)

    # SPMD-uniform chunk schedule: max over cores per (tile, half)
    ch = np.zeros((C, T, 2), np.int64)
    for g in range(n_groups):
        c, rem = divmod(g, T * 2)
        t, h = divmod(rem, 2)
        ch[c, t, h] = (len(uniq[g]) + P - 1) // P
    sched = np.maximum(ch.max(axis=0), 1)  # [T, 2]
    SCH = int(sched.sum())

    flat_sched = sched.reshape(-1)
    chunk_off = np.zeros(T * 2 + 1, np.int64)
    np.cumsum(flat_sched, out=chunk_off[1:])

    per_core = []
    for c in range(C):
        idx_cols = np.zeros((P, SCH * 8), np.int16)
        cap = E // C + 4 * SCH * P + E // 16
        flat_rows = np.zeros(cap, np.int64)
        flat_dcol = np.zeros(cap, np.int64)
        flat_vals = np.zeros(cap, np.float32)
        n_e = 0
        for t in range(T):
            for h in range(2):
                g = (c * T + t) * 2 + h
                nch = int(sched[t, h])
                K = nch * P
                u = uniq[g]
                u_pad = np.zeros(K, np.int64)
                u_pad[: len(u)] = u
                wrapped = u_pad.astype(np.int16).reshape(nch * 8, 16).T
                co = int(chunk_off[t * 2 + h])
                idx_cols[:, co * 8 : (co + nch) * 8] = np.tile(wrapped, (8, 1))
                lo, hi = bounds[g], bounds[g + 1]
                ne = hi - lo
                flat_rows[n_e : n_e + ne] = co * P + invs[g]
                flat_dcol[n_e : n_e + ne] = dcol_s[lo:hi]
                flat_vals[n_e : n_e + ne] = vals_s[lo:hi]
                n_e += ne
        flat = flat_rows[:n_e] * P + flat_dcol[:n_e]
        s_core = np.bincount(flat, weights=flat_vals[:n_e], minlength=SCH * P * P)
        s_core = s_core.astype(np.float32).reshape(SCH, P, P)
        smat = np.ascontiguousarray(s_core.transpose(1, 0, 2)).reshape(P, SCH * P)

        xT = np.zeros((P, NPAD), np.float32)
        xT[:, :NPC] = x[c * NPC : (c + 1) * NPC].T
        dv = np.zeros(NPAD, np.float32)
        dv[:NPC] = dinv[c * NPC : (c + 1) * NPC]
        dinv_tiles = np.ascontiguousarray(dv.reshape(T, P).T)  # [P, T]

        per_core.append({"xT": xT, "dinv": dinv_tiles, "idx": idx_cols, "smat": smat})

    plan = {
        "N": N, "D": D, "E": E, "C": C, "NPC": NPC, "NPAD": NPAD, "T": T,
        "TA": TA, "TB": TB, "BLKA": BLKA, "BLKB": BLKB,
        "FULLA": FULLA, "FULLB": FULLB, "SCH": SCH,
        "sched": sched, "chunk_off": chunk_off,
    }
    return plan, per_core


def _build(plan):
    T, TA, TB = plan["T"], plan["TA"], plan["TB"]
    NPAD = plan["NPAD"]
    BLKA, BLKB = plan["BLKA"], plan["BLKB"]
    FULLA, FULLB = plan["FULLA"], plan["FULLB"]
    SCH = plan["SCH"]
    sched = plan["sched"]
    chunk_off = plan["chunk_off"]
    maxch = int(sched.max())

    nc = bacc.Bacc(
        get_trn_type() or "TRN2",
        target_bir_lowering=False,
        debug=False,
        num_devices=N_CORES,
    )
    xT_in = nc.dram_tensor("xT", [P, NPAD], f32, kind="ExternalInput").ap()
    w1_in = nc.dram_tensor("w1", [P, P], f32, kind="ExternalInput").ap()
    w2_in = nc.dram_tensor("w2", [P, P], f32, kind="ExternalInput").ap()
    lin1_in = nc.dram_tensor("lin1", [P, P], f32, kind="ExternalInput").ap()
    lin2_in = nc.dram_tensor("lin2", [P, P], f32, kind="ExternalInput").ap()
    b1_in = nc.dram_tensor("b1", [P, 1], f32, kind="ExternalInput").ap()
    b2_in = nc.dram_tensor("b2", [P, 1], f32, kind="ExternalInput").ap()
    linb_in = nc.dram_tensor("linb", [P, P], f32, kind="ExternalInput").ap()
    dinv_in = nc.dram_tensor("dinv", [P, T], f32, kind="ExternalInput").ap()
    idx_in = nc.dram_tensor("idx", [P, SCH * 8], i16, kind="ExternalInput").ap()
    smat_in = nc.dram_tensor("smat", [P, SCH * P], f32, kind="ExternalInput").ap()
    out_ap = nc.dram_tensor("out", [NPAD, P], f32, kind="ExternalOutput").ap()
    out_v = out_ap.rearrange("(t p) f -> p t f", p=P)

    nc.gpsimd.load_library(mlp)

    with tile.TileContext(nc) as tc:
        with (
            tc.tile_pool(name="dram", bufs=1, space="DRAM") as dram,
            tc.tile_pool(name="consts", bufs=1) as consts,
            tc.tile_pool(name="xTp", bufs=1) as xTp,
            tc.tile_pool(name="stages", bufs=1) as stages,
            tc.tile_pool(name="msg", bufs=5) as msgp,
            tc.tile_pool(name="smat", bufs=5) as smatp,
            tc.tile_pool(name="diag", bufs=3) as diagp,
            tc.tile_pool(name="otile", bufs=3) as otilep,
            tc.tile_pool(name="ps_phase", bufs=4, space="PSUM") as psphase,
            tc.tile_pool(name="ps_agg", bufs=4, space="PSUM") as psagg,
        ):
            xT = xTp.tile([P, NPAD], f32)
            nc.sync.dma_start(xT[:], xT_in[:])
            w1 = consts.tile([P, P], f32, tag="w1")
            nc.sync.dma_start(w1[:], w1_in[:])
            w2 = consts.tile([P, P], f32, tag="w2")
            nc.sync.dma_start(w2[:], w2_in[:])
            lin1 = consts.tile([P, P], f32, tag="lin1")
            nc.sync.dma_start(lin1[:], lin1_in[:])
            lin2 = consts.tile([P, P], f32, tag="lin2")
            nc.sync.dma_start(lin2[:], lin2_in[:])
            b1 = consts.tile([P, 1], f32, tag="b1")
            nc.sync.dma_start(b1[:], b1_in[:])
            b2 = consts.tile([P, 1], f32, tag="b2")
            nc.sync.dma_start(b2[:], b2_in[:])
            linb = consts.tile([P, P], f32, tag="linb")
            nc.sync.dma_start(linb[:], linb_in[:])
            dinv = consts.tile([P, T], f32, tag="dinv")
            nc.sync.dma_start(dinv[:], dinv_in[:])
            idx_sb = consts.tile([P, SCH * 8], i16, tag="idx")
            nc.sync.dma_start(idx_sb[:], idx_in[:])
            ident = consts.tile([P, P], f32, tag="ident")
            make_identity(nc, ident[:])

            # per-block stage tiles: [P, BLKA] and [P, BLKB]
            def blk_pair(tag):
                a = stages.tile([P, BLKA], f32, tag=f"{tag}A", name=f"{tag}A")
                b = stages.tile([P, BLKB], f32, tag=f"{tag}B", name=f"{tag}B")
                return [a, b]

            gstage = blk_pair("gstage")
            x1T = blk_pair("x1T")
            x2T = blk_pair("x2T")

            g_loc = [[None, None], [None, None]]
            g_full = [[None, None], [None, None]]
            for layer in range(2):
                for h, (blkrows, fullrows) in enumerate([(BLKA, FULLA), (BLKB, FULLB)]):
                    g_loc[layer][h] = dram.tile(
                        [blkrows, P], f32, tag=f"gloc{layer}{h}", name=f"gloc{layer}{h}"
                    )
                    g_full[layer][h] = dram.tile(
                        [fullrows, P], f32, tag=f"gfull{layer}{h}", name=f"gfull{layer}{h}"
                    )

            def loc_tile(t):
                """(stage-half h, column-tile index within that half)"""
                return (0, t) if t < TA else (1, t - TA)

            def phase_g_block(src_stages, w_tile, layer, h):
                t0 = 0 if h == 0 else TA
                nt = TA if h == 0 else TB
                gs = gstage[h]
                for i in range(nt):
                    t = t0 + i
                    hh, ii = loc_tile(t)
                    ps = psphase.tile([P, P], f32, tag="ps_phase", name="psph")
                    nc.tensor.matmul(
                        ps[:], lhsT=src_stages[hh][:, bass.ts(ii, P)], rhs=w_tile[:],
                        start=True, stop=True,
                    )
                    nc.vector.tensor_scalar(
                        gs[:, bass.ts(i, P)], ps[:],
                        dinv[:, t : t + 1], None, mybir.AluOpType.mult,
                    )
                gl = g_loc[layer][h]
                nc.sync.dma_start(gl[:].rearrange("(t p) f -> p t f", p=P), gs[:])
                nc.gpsimd.collective_compute(
                    "AllGather",
                    mybir.AluOpType.bypass,
                    replica_groups=[list(range(N_CORES))],
                    ins=[gl.opt()],
                    outs=[g_full[layer][h].opt()],
                )

            partial = xTp.tile([P, NPAD], f32, tag="xT", name="partial")

            def layer_agg(layer, xout, bias, hooks=None, post_tile=None):
                # pass 1: block-A chunks only -> partial (so the stream only
                # needs table A, which is AllGather'd first)
                for t in range(T):
                    c0 = int(sched[t, 0])
                    K = c0 * P
                    co = int(chunk_off[t * 2])
                    msg = msgp.tile([P, maxch, P], f32, tag="msg", name="msg")
                    s_sb = smatp.tile([P, maxch * P], f32, tag="smat", name="ssb")
                    nc.sync.dma_start(
                        s_sb[:, 0 : c0 * P], smat_in[:, co * P : (co + c0) * P]
                    )
                    nc.gpsimd.dma_gather(
                        msg[:, 0:c0, :],
                        g_full[layer][0][:],
                        idx_sb[:, co * 8 : (co + c0) * 8],
                        K, K, P,
                        single_packet=False,
                    )
                    ps = psagg.tile([P, P], f32, tag="ps_agg", name="psagg")
                    for j in range(c0):
                        nc.tensor.matmul(
                            ps[:], lhsT=msg[:, j, :],
                            rhs=s_sb[:, bass.ts(j, P)],
                            start=(j == 0), stop=(j == c0 - 1),
                        )
                    nc.vector.tensor_copy(out=partial[:, bass.ts(t, P)], in_=ps[:])
                # pass 2: block-B chunks + self-loop, add partial, relu
                for t in range(T):
                    if hooks and t in hooks:
                        hooks[t]()
                    c1 = int(sched[t, 1])
                    K = c1 * P
                    co = int(chunk_off[t * 2 + 1])
                    msg = msgp.tile([P, maxch, P], f32, tag="msg", name="msg")
                    s_sb = smatp.tile([P, maxch * P], f32, tag="smat", name="ssb")
                    nc.sync.dma_start(
                        s_sb[:, 0 : c1 * P], smat_in[:, co * P : (co + c1) * P]
                    )
                    nc.gpsimd.dma_gather(
                        msg[:, 0:c1, :],
                        g_full[layer][1][:],
                        idx_sb[:, co * 8 : (co + c1) * 8],
                        K, K, P,
                        single_packet=False,
                    )
                    diag = diagp.tile([P, P], f32, tag="diag", name="diag")
                    nc.vector.tensor_scalar(
                        diag[:], ident[:], dinv[:, t : t + 1], None,
                        mybir.AluOpType.mult,
                    )
                    ps = psagg.tile([P, P], f32, tag="ps_agg", name="psagg")
                    for j in range(c1):
                        nc.tensor.matmul(
                            ps[:], lhsT=msg[:, j, :],
                            rhs=s_sb[:, bass.ts(j, P)],
                            start=(j == 0), stop=False,
                        )
                    hh, ii = loc_tile(t)
                    nc.tensor.matmul(
                        ps[:], lhsT=gstage[hh][:, bass.ts(ii, P)], rhs=diag[:],
                        start=(c1 == 0), stop=True,
                    )
                    nc.vector.tensor_tensor(
                        out=ps[:], in0=ps[:], in1=partial[:, bass.ts(t, P)],
                        op=mybir.AluOpType.add,
                    )
                    nc.scalar.activation(
                        xout[hh][:, bass.ts(ii, P)], ps[:],
                        mybir.ActivationFunctionType.Relu, bias=bias[:],
                    )
                    if post_tile is not None:
                        post_tile(t)

            xT_stages = [xT[:, 0:BLKA], xT[:, BLKA:NPAD]]

            class _W:  # tiny adapter so phase_g_block can slice uniformly
                def __init__(self, aps):
                    self.aps = aps
                def __getitem__(self, h):
                    return self.aps[h]

            phase_g_block(_W(xT_stages), w1, 0, 0)
            phase_g_block(_W(xT_stages), w1, 0, 1)
            # layer-2 block-A table (x1T-A ready after L1 tile TA-1) is
            # AllGather'd mid-stream: its gpsimd trigger is emitted between
            # L1 gathers so the collective overlaps the gather stream.
            layer_agg(
                0, x1T, b1,
                hooks={TA + 2: lambda: phase_g_block(x1T, w2, 1, 0)},
            )
            phase_g_block(x1T, w2, 1, 1)

            def final_tile(t):
                hh, ii = loc_tile(t)
                ps = psphase.tile([P, P], f32, tag="ps_phase", name="psph")
                nc.tensor.matmul(
                    ps[:], lhsT=x1T[hh][:, bass.ts(ii, P)], rhs=lin1[:],
                    start=True, stop=False,
                )
                nc.tensor.matmul(
                    ps[:], lhsT=x2T[hh][:, bass.ts(ii, P)], rhs=lin2[:],
                    start=False, stop=True,
                )
                ot = otilep.tile([P, P], f32, tag="otile", name="otile")
                nc.vector.tensor_tensor(
                    out=ot[:], in0=ps[:], in1=linb[:], op=mybir.AluOpType.add
                )
                nc.sync.dma_start(out_v[:, t, :], ot[:])

            layer_agg(1, x2T, b2, post_tile=final_tile)

    nc.compile()
    return nc


def kernel(x, edge_index, W1, b1, W2, b2, lin_W, lin_b):
    x = np.asarray(x, np.float32)
    edge_index = np.asarray(edge_index)
    W1 = np.asarray(W1, np.float32)
    W2 = np.asarray(W2, np.float32)
    b1 = np.asarray(b1, np.float32)
    b2 = np.asarray(b2, np.float32)
    lin_W = np.asarray(lin_W, np.float32)
    lin_b = np.asarray(lin_b, np.float32)

    plan, per_core = _preprocess(x, edge_index)
    nc = _build(plan)

    N, D, C, NPC = plan["N"], plan["D"], plan["C"], plan["NPC"]
    in_maps = []
    for c in range(C):
        pc = per_core[c]
        in_maps.append(
            {
                "xT": pc["xT"],
                "w1": W1,
                "w2": W2,
                "lin1": np.ascontiguousarray(lin_W[:D]),
                "lin2": np.ascontiguousarray(lin_W[D:]),
                "b1": b1[:, None].astype(np.float32),
                "b2": b2[:, None].astype(np.float32),
                "linb": np.tile(lin_b, (P, 1)).astype(np.float32),
                "dinv": pc["dinv"],
                "idx": pc["idx"],
                "smat": pc["smat"],
            }
        )

    last_err = None
    for _attempt in range(3):
        try:
            res = run_bass_kernel_spmd(nc, in_maps, list(range(C)))
            break
        except Exception as e:  # transient NRT device wedges happen
            last_err = e
    else:
        raise last_err

    out = np.empty((N, D), np.float32)
    for c in range(C):
        out[c * NPC : (c + 1) * NPC] = res.results[c]["out"][:NPC]
    return out


# revision 11
# speedup vs baseline: 1.1334x; 1.0354x over previous
"""2-layer GCN + JumpingKnowledge(cat) + Linear on 8 Trainium2 NeuronCores.

Strategy (graph-parallel, nodes sharded 6250/core):
  - g = dinv * (x @ W) computed per-core (TensorE + DVE), written to HBM,
    AllGather'd so every core holds the full node-feature table.
  - Message passing per destination tile (128 dsts): dma_gather pulls the
    unique source rows (fp32, 512B each) from the table; aggregation is a
    chain of TensorE matmuls  psum[feat,dst] += M_chunk^T @ S_chunk  where
    S (host-built, fp32) carries the symmetric-norm coefficients dinv[dst]
    (multiplicity-summed, dedup'd sources).  Self-loops use the local g
    tile against diag(dinv) - no gather needed.
  - relu(+bias) on ScalarE writes the transposed activations x^T directly,
    which feeds the next layer's matmuls without any transposes.
  - Final: out = x1 @ lin_W[:D] + x2 @ lin_W[D:] (+ lin_b) per tile.

dma_gather needs int16 indices, so the global table is built as two block
tables (each core contributes rows [0,3200) to table A and [3200,6272) to
table B).  Each block is AllGather'd separately so the collectives overlap
the gather stream of the previous work (the single Q7 descriptor-generation
core is the critical path at ~8.4ns/row; everything else hides under it).
"""
import numpy as np

import concourse.bass as bass
import concourse.bacc as bacc
import concourse.mybir as mybir
import concourse.tile as tile
from concourse._compat import get_trn_type
from concourse.bass_utils import run_bass_kernel_spmd
from concourse.library_config import mlp
from concourse.masks import make_identity

P = 128
N_CORES = 8

f32 = mybir.dt.float32
i16 = mybir.dt.int16


def _preprocess(x, edge_index):
    """Host-side (numpy): shard, block-split, dedup, build gather indices
    and the norm-coefficient matrices S."""
    N, D = x.shape
    assert D == P
    E = edge_index.shape[1]
    C = N_CORES
    NPC = (N + C - 1) // C
    assert NPC * C == N, "node count must split evenly across cores"
    NPAD = ((NPC + P - 1) // P) * P
    T = NPAD // P
    TA = (T + 1) // 2          # tiles in block A
    TB = T - TA
    BLKA, BLKB = TA * P, TB * P
    FULLA, FULLB = C * BLKA, C * BLKB
    assert FULLA <= 32768 and FULLB <= 32768

    src = edge_index[0].astype(np.int64)
    dst = edge_index[1].astype(np.int64)

    deg = np.bincount(dst, minlength=N).astype(np.float32) + 1.0
    dinv = (1.0 / np.sqrt(deg)).astype(np.float32)

    off = src % NPC
    blk = (off >= BLKA).astype(np.int64)
    row_in_blk = np.where(blk == 0, (src // NPC) * BLKA + off,
                          (src // NPC) * BLKB + (off - BLKA))
    core = dst // NPC
    dloc = dst % NPC
    tl = dloc // P
    dcol = dloc % P
    vals = dinv[dst]

    gkey = (core * T + tl) * 2 + blk
    order = np.lexsort((row_in_blk, gkey))
    gkey_s = gkey[order]
    rows_s = row_in_blk[order]
    dcol_s = dcol[order]
    vals_s = vals[order]
    n_groups = C * T * 2
    bounds = np.searchsorted(gkey_s, np.arange(n_groups + 1))

    uniq = [None] * n_groups
    invs = [None] * n_groups
    for g in range(n_groups):
        lo, hi = bounds[g], bounds[g + 1]
        r = rows_s[lo:hi]
        u = np.unique(r)
        uniq[g] = u
        invs[g] = np.searchsorted(u, r)

    # SPMD-uniform chunk schedule: max over cores per (tile, half)
    ch = np.zeros((C, T, 2), np.int64)
    for g in range(n_groups):
        c, rem = divmod(g, T * 2)
        t, h = divmod(rem, 2)
        ch[c, t, h] = (len(uniq[g]) + P - 1) // P
    sched = np.maximum(ch.max(axis=0), 1)  # [T, 2]
    SCH = int(sched.sum())

    flat_sched = sched.reshape(-1)
    chunk_off = np.zeros(T * 2 + 1, np.int64)
    np.cumsum(flat_sched, out=chunk_off[1:])

    per_core = []
    for c in range(C):
        idx_cols = np.zeros((P, SCH * 8), np.int16)
        cap = E // C + 4 * SCH * P + E // 16
        flat_rows = np.zeros(cap, np.int64)
        flat_dcol = np.zeros(cap, np.int64)
        flat_vals = np.zeros(cap, np.float32)
        n_e = 0
        for t in range(T):
            for h in range(2):
                g = (c * T + t) * 2 + h
                nch = int(sched[t, h])
                K = nch * P
                u = uniq[g]
                u_pad = np.zeros(K, np.int64)
                u_pad[: len(u)] = u
                wrapped = u_pad.astype(np.int16).reshape(nch * 8, 16).T
                co = int(chunk_off[t * 2 + h])
                idx_cols[:, co * 8 : (co + nch) * 8] = np.tile(wrapped, (8, 1))
                lo, hi = bounds[g], bounds[g + 1]
                ne = hi - lo
                flat_rows[n_e : n_e + ne] = co * P + invs[g]
                flat_dcol[n_e : n_e + ne] = dcol_s[lo:hi]
                flat_vals[n_e : n_e + ne] = vals_s[lo:hi]
                n_e += ne
        flat = flat_rows[:n_e] * P + flat_dcol[:n_e]
        s_core = np.bincount(flat, weights=flat_vals[:n_e], minlength=SCH * P * P)
        s_core = s_core.astype(np.float32).reshape(SCH, P, P)
        smat = np.ascontiguousarray(s_core.transpose(1, 0, 2)).reshape(P, SCH * P)

        xT = np.zeros((P, NPAD), np.float32)
        xT[:, :NPC] = x[c * NPC : (c + 1) * NPC].T
        dv = np.zeros(NPAD, np.float32)
        dv[:NPC] = dinv[c * NPC : (c + 1) * NPC]
        dinv_tiles = np.ascontiguousarray(dv.reshape(T, P).T)  # [P, T]

        per_core.append({"xT": xT, "dinv": dinv_tiles, "idx": idx_cols, "smat": smat})

    plan = {
        "N": N, "D": D, "E": E, "C": C, "NPC": NPC, "NPAD": NPAD, "T": T,
        "TA": TA, "TB": TB, "BLKA": BLKA, "BLKB": BLKB,
        "FULLA": FULLA, "FULLB": FULLB, "SCH": SCH,
        "sched": sched, "chunk_off": chunk_off,
    }
    return plan, per_core


def _build(plan):
    T, TA, TB = plan["T"], plan["TA"], plan["TB"]
    NPAD = plan["NPAD"]
    BLKA, BLKB = plan["BLKA"], plan["BLKB"]
    FULLA, FULLB = plan["FULLA"], plan["FULLB"]
    SCH = plan["SCH"]
    sched = plan["sched"]
    chunk_off = plan["chunk_off"]
    maxch = int(sched.max())

    nc = bacc.Bacc(
        get_trn_type() or "TRN2",
        target_bir_lowering=False,
        debug=False,
        num_devices=N_CORES,
    )
    xT_in = nc.dram_tensor("xT", [P, NPAD], f32, kind="ExternalInput").ap()
    w1_in = nc.dram_tensor("w1", [P, P], f32, kind="ExternalInput").ap()
    w2_in = nc.dram_tensor("w2", [P, P], f32, kind="ExternalInput").ap()
    lin1_in = nc.dram_tensor("lin1", [P, P], f32, kind="ExternalInput").ap()
    lin2_in = nc.dram_tensor("lin2", [P, P], f32, kind="ExternalInput").ap()
    b1_in = nc.dram_tensor("b1", [P, 1], f32, kind="ExternalInput").ap()
    b2_in = nc.dram_tensor("b2", [P, 1], f32, kind="ExternalInput").ap()
    linb_in = nc.dram_tensor("linb", [P, P], f32, kind="ExternalInput").ap()
    dinv_in = nc.dram_tensor("dinv", [P, T], f32, kind="ExternalInput").ap()
    idx_in = nc.dram_tensor("idx", [P, SCH * 8], i16, kind="ExternalInput").ap()
    smat_in = nc.dram_tensor("smat", [P, SCH * P], f32, kind="ExternalInput").ap()
    out_ap = nc.dram_tensor("out", [NPAD, P], f32, kind="ExternalOutput").ap()
    out_v = out_ap.rearrange("(t p) f -> p t f", p=P)

    nc.gpsimd.load_library(mlp)

    with tile.TileContext(nc) as tc:
        with (
            tc.tile_pool(name="dram", bufs=1, space="DRAM") as dram,
            tc.tile_pool(name="consts", bufs=1) as consts,
            tc.tile_pool(name="xTp", bufs=1) as xTp,
            tc.tile_pool(name="stages", bufs=1) as stages,
            tc.tile_pool(name="msg", bufs=8) as msgp,
            tc.tile_pool(name="smat", bufs=8) as smatp,
            tc.tile_pool(name="diag", bufs=3) as diagp,
            tc.tile_pool(name="otile", bufs=3) as otilep,
            tc.tile_pool(name="ps_phase", bufs=4, space="PSUM") as psphase,
            tc.tile_pool(name="ps_agg", bufs=4, space="PSUM") as psagg,
        ):
            xT = xTp.tile([P, NPAD], f32)
            nc.sync.dma_start(xT[:], xT_in[:])
            w1 = consts.tile([P, P], f32, tag="w1")
            nc.sync.dma_start(w1[:], w1_in[:])
            w2 = consts.tile([P, P], f32, tag="w2")
            nc.sync.dma_start(w2[:], w2_in[:])
            lin1 = consts.tile([P, P], f32, tag="lin1")
            nc.sync.dma_start(lin1[:], lin1_in[:])
            lin2 = consts.tile([P, P], f32, tag="lin2")
            nc.sync.dma_start(lin2[:], lin2_in[:])
            b1 = consts.tile([P, 1], f32, tag="b1")
            nc.sync.dma_start(b1[:], b1_in[:])
            b2 = consts.tile([P, 1], f32, tag="b2")
            nc.sync.dma_start(b2[:], b2_in[:])
            linb = consts.tile([P, P], f32, tag="linb")
            nc.sync.dma_start(linb[:], linb_in[:])
            dinv = consts.tile([P, T], f32, tag="dinv")
            nc.sync.dma_start(dinv[:], dinv_in[:])
            idx_sb = consts.tile([P, SCH * 8], i16, tag="idx")
            nc.sync.dma_start(idx_sb[:], idx_in[:])
            ident = consts.tile([P, P], f32, tag="ident")
            make_identity(nc, ident[:])

            # per-block stage tiles: [P, BLKA] and [P, BLKB]
            def blk_pair(tag):
                a = stages.tile([P, BLKA], f32, tag=f"{tag}A", name=f"{tag}A")
                b = stages.tile([P, BLKB], f32, tag=f"{tag}B", name=f"{tag}B")
                return [a, b]

            gstage = blk_pair("gstage")
            x1T = blk_pair("x1T")
            x2T = blk_pair("x2T")

            g_loc = [[None, None], [None, None]]
            g_full = [[None, None], [None, None]]
            for layer in range(2):
                for h, (blkrows, fullrows) in enumerate([(BLKA, FULLA), (BLKB, FULLB)]):
                    g_loc[layer][h] = dram.tile(
                        [blkrows, P], f32, tag=f"gloc{layer}{h}", name=f"gloc{layer}{h}"
                    )
                    g_full[layer][h] = dram.tile(
                        [fullrows, P], f32, tag=f"gfull{layer}{h}", name=f"gfull{layer}{h}"
                    )

            def loc_tile(t):
                """(stage-half h, column-tile index within that half)"""
                return (0, t) if t < TA else (1, t - TA)

            def phase_g_block(src_stages, w_tile, layer, h):
                t0 = 0 if h == 0 else TA
                nt = TA if h == 0 else TB
                gs = gstage[h]
                for i in range(nt):
                    t = t0 + i
                    hh, ii = loc_tile(t)
                    ps = psphase.tile([P, P], f32, tag="ps_phase", name="psph")
                    nc.tensor.matmul(
                        ps[:], lhsT=src_stages[hh][:, bass.ts(ii, P)], rhs=w_tile[:],
                        start=True, stop=True,
                    )
                    nc.vector.tensor_scalar(
                        gs[:, bass.ts(i, P)], ps[:],
                        dinv[:, t : t + 1], None, mybir.AluOpType.mult,
                    )
                gl = g_loc[layer][h]
                nc.sync.dma_start(gl[:].rearrange("(t p) f -> p t f", p=P), gs[:])
                nc.gpsimd.collective_compute(
                    "AllGather",
                    mybir.AluOpType.bypass,
                    replica_groups=[list(range(N_CORES))],
                    ins=[gl.opt()],
                    outs=[g_full[layer][h].opt()],
                )

            partial = xTp.tile([P, NPAD], f32, tag="xT", name="partial")

            def layer_agg(layer, xout, bias, hooks=None, post_tile=None):
                # pass 1: block-A chunks only -> partial (so the stream only
                # needs table A, which is AllGather'd first)
                for t in range(T):
                    c0 = int(sched[t, 0])
                    K = c0 * P
                    co = int(chunk_off[t * 2])
                    msg = msgp.tile([P, maxch, P], f32, tag="msg", name="msg")
                    s_sb = smatp.tile([P, maxch * P], f32, tag="smat", name="ssb")
                    nc.sync.dma_start(
                        s_sb[:, 0 : c0 * P], smat_in[:, co * P : (co + c0) * P]
                    )
                    nc.gpsimd.dma_gather(
                        msg[:, 0:c0, :],
                        g_full[layer][0][:],
                        idx_sb[:, co * 8 : (co + c0) * 8],
                        K, K, P,
                        single_packet=False,
                    )
                    ps = psagg.tile([P, P], f32, tag="ps_agg", name="psagg")
                    for j in range(c0):
                        nc.tensor.matmul(
                            ps[:], lhsT=msg[:, j, :],
                            rhs=s_sb[:, bass.ts(j, P)],
                            start=(j == 0), stop=(j == c0 - 1),
                        )
                    nc.vector.tensor_copy(out=partial[:, bass.ts(t, P)], in_=ps[:])
                # pass 2: block-B chunks + self-loop, add partial, relu
                for t in range(T):
                    if hooks and t in hooks:
                        hooks[t]()
                    c1 = int(sched[t, 1])
                    K = c1 * P
                    co = int(chunk_off[t * 2 + 1])
                    msg = msgp.tile([P, maxch, P], f32, tag="msg", name="msg")
                    s_sb = smatp.tile([P, maxch * P], f32, tag="smat", name="ssb")
                    nc.sync.dma_start(
                        s_sb[:, 0 : c1 * P], smat_in[:, co * P : (co + c1) * P]
                    )
                    nc.gpsimd.dma_gather(
                        msg[:, 0:c1, :],
                        g_full[layer][1][:],
                        idx_sb[:, co * 8 : (co + c1) * 8],
                        K, K, P,
                        single_packet=False,
                    )
                    diag = diagp.tile([P, P], f32, tag="diag", name="diag")
                    nc.vector.tensor_scalar(
                        diag[:], ident[:], dinv[:, t : t + 1], None,
                        mybir.AluOpType.mult,
                    )
                    ps = psagg.tile([P, P], f32, tag="ps_agg", name="psagg")
                    for j in range(c1):
                        nc.tensor.matmul(
                            ps[:], lhsT=msg[:, j, :],
                            rhs=s_sb[:, bass.ts(j, P)],
                            start=(j == 0), stop=False,
                        )
                    hh, ii = loc_tile(t)
                    nc.tensor.matmul(
                        ps[:], lhsT=gstage[hh][:, bass.ts(ii, P)], rhs=diag[:],
                        start=(c1 == 0), stop=True,
                    )
                    nc.vector.tensor_tensor(
                        out=ps[:], in0=ps[:], in1=partial[:, bass.ts(t, P)],
                        op=mybir.AluOpType.add,
                    )
                    nc.scalar.activation(
                        xout[hh][:, bass.ts(ii, P)], ps[:],
                        mybir.ActivationFunctionType.Relu, bias=bias[:],
                    )
                    if post_tile is not None:
                        post_tile(t)

            xT_stages = [xT[:, 0:BLKA], xT[:, BLKA:NPAD]]

            class _W:  # tiny adapter so phase_g_block can slice uniformly
                def __init__(self, aps):
                    self.aps = aps
                def __getitem__(self, h):
                    return self.aps[h]

            phase_g_block(_W(xT_stages), w1, 0, 0)
            phase_g_block(_W(xT_stages), w1, 0, 1)
            # layer-2 block-A table (x1T-A ready after L1 tile TA-1) is
            # AllGather'd mid-stream: its gpsimd trigger is emitted between
            # L1 gathers so the collective overlaps the gather stream.
            layer_agg(
                0, x1T, b1,
                hooks={TA + 2: lambda: phase_g_block(x1T, w2, 1, 0)},
            )
            phase_g_block(x1T, w2, 1, 1)

            def final_tile(t):
                hh, ii = loc_tile(t)
                ps = psphase.tile([P, P], f32, tag="ps_phase", name="psph")
                nc.tensor.matmul(
                    ps[:], lhsT=x1T[hh][:, bass.ts(ii, P)], rhs=lin1[:],
                    start=True, stop=False,
                )
                nc.tensor.matmul(
                    ps[:], lhsT=x2T[hh][:, bass.ts(ii, P)], rhs=lin2[:],
                    start=False, stop=True,
                )
                ot = otilep.tile([P, P], f32, tag="otile", name="otile")
                nc.vector.tensor_tensor(
                    out=ot[:], in0=ps[:], in1=linb[:], op=mybir.AluOpType.add
                )
                nc.sync.dma_start(out_v[:, t, :], ot[:])

            layer_agg(1, x2T, b2, post_tile=final_tile)

    nc.compile()
    return nc


def kernel(x, edge_index, W1, b1, W2, b2, lin_W, lin_b):
    x = np.asarray(x, np.float32)
    edge_index = np.asarray(edge_index)
    W1 = np.asarray(W1, np.float32)
    W2 = np.asarray(W2, np.float32)
    b1 = np.asarray(b1, np.float32)
    b2 = np.asarray(b2, np.float32)
    lin_W = np.asarray(lin_W, np.float32)
    lin_b = np.asarray(lin_b, np.float32)

    plan, per_core = _preprocess(x, edge_index)
    nc = _build(plan)

    N, D, C, NPC = plan["N"], plan["D"], plan["C"], plan["NPC"]
    in_maps = []
    for c in range(C):
        pc = per_core[c]
        in_maps.append(
            {
                "xT": pc["xT"],
                "w1": W1,
                "w2": W2,
                "lin1": np.ascontiguousarray(lin_W[:D]),
                "lin2": np.ascontiguousarray(lin_W[D:]),
                "b1": b1[:, None].astype(np.float32),
                "b2": b2[:, None].astype(np.float32),
                "linb": np.tile(lin_b, (P, 1)).astype(np.float32),
                "dinv": pc["dinv"],
                "idx": pc["idx"],
                "smat": pc["smat"],
            }
        )

    last_err = None
    for _attempt in range(3):
        try:
            res = run_bass_kernel_spmd(nc, in_maps, list(range(C)))
            break
        except Exception as e:  # transient NRT device wedges happen
            last_err = e
    else:
        raise last_err

    out = np.empty((N, D), np.float32)
    for c in range(C):
        out[c * NPC : (c + 1) * NPC] = res.results[c]["out"][:NPC]
    return out
